# revision 1
# baseline (speedup 1.0000x reference)
"""GATv2 2-layer GNN on 8 Trainium2 NeuronCores (Bass/Tile, edge-parallel).

Sharding: edges sorted by dst node, dst-range sharded across 8 cores
(core k owns dst nodes [1250k, 1250(k+1))), so the per-dst segment
softmax and aggregation are fully core-local. Node-side projections for
layer 1 are computed replicated (xl1 for all nodes; xr1 for the own
slice). Between layers only the 32-wide layer-2 projections are
exchanged with a single AllGather.
"""
import sys
sys.path.insert(0, "/opt/trn_rl_repo")

import numpy as np
import ml_dtypes

import concourse.bass as bass
import concourse.bacc as bacc
import concourse.tile as tile
from concourse import mybir
from concourse.bass_utils import run_bass_kernel_spmd

BF16 = ml_dtypes.bfloat16

N, E, F = 10000, 80000, 128
H1, C1 = 8, 256
D1 = H1 * C1          # 2048
D2 = 32               # layer-2 out (1 head)
NEG = 0.2
M = 8                 # cores
NPC = N // M          # 1250 nodes per core
GN = 125              # dst nodes per group
G = NPC // GN         # 10 groups per core
P = 128

dt = mybir.dt


def _build_program(CH, phases="ABC"):
    """Build the SPMD Bass program. CH = chunks per group (incl. 1 self chunk)."""
    L = G * CH * P  # edge-stream length per core
    nc = bacc.Bacc("TRN2", target_bir_lowering=False, debug=False, num_devices=M)

    # ---- external inputs (per-core data differs only for edge/slice tensors)
    ei = {}
    def EIN(name, shape, dtype):
        ei[name] = nc.dram_tensor(name, list(shape), dtype, kind="ExternalInput")
        return ei[name]

    xT    = EIN("xT",    (P, N),      dt.bfloat16)   # x transposed (replicated)
    xsT   = EIN("xsT",   (P, G * P),  dt.bfloat16)   # own-slice cols, group padded
    wl1   = EIN("wl1",   (F, D1),     dt.bfloat16)
    wr1   = EIN("wr1",   (F, D1),     dt.bfloat16)
    we1   = EIN("we1",   (F, D1),     dt.bfloat16)
    att1r = EIN("att1r", (P, D1),     dt.bfloat16)   # att1 row-replicated
    wl2   = EIN("wl2",   (P, 16 * D2), dt.bfloat16)  # [p, k*32+c] = Wl2[k*128+p, c]
    wr2   = EIN("wr2",   (P, 16 * D2), dt.bfloat16)
    we2   = EIN("we2",   (F, D2),     dt.bfloat16)
    att2r = EIN("att2r", (P, D2),     dt.bfloat16)
    eaT   = EIN("eaT",   (F, L),      dt.bfloat16)   # edge_attr^T, sorted+padded
    eaN   = EIN("eaN",   (L, F),      dt.bfloat16)   # edge_attr row-major
    s01   = EIN("s01",   (L, P),      dt.bfloat16)   # one-hot dst selector
    srci  = EIN("srci",  (L, 1),      dt.int32)      # global src node id
    dstpi = EIN("dstpi", (L, 1),      dt.int32)      # group-padded local dst id
    dstgi = EIN("dstgi", (L, 1),      dt.int32)      # global dst id
    invc  = EIN("invc",  (G * P, 1),  dt.float32)    # 1/max(cnt,1) per dst node

    out = nc.dram_tensor("out", [NPC, D2], dt.float32, kind="ExternalOutput")

    # ---- DRAM scratch
    xl1_tab = nc.dram_tensor("xl1_tab", [N, D1], dt.bfloat16)
    xr1_sl  = nc.dram_tensor("xr1_sl", [G * P, D1], dt.bfloat16)
    xlr2    = nc.dram_tensor("xlr2", [NPC, 2 * D2], dt.float32)
    ag_out  = nc.dram_tensor("ag_out", [N, 2 * D2], dt.float32, addr_space="Shared")
    xl2_tab = nc.dram_tensor("xl2_tab", [N, D2], dt.float32)
    xr2_tab = nc.dram_tensor("xr2_tab", [N, D2], dt.float32)

    AF = mybir.ActivationFunctionType
    ALU = mybir.AluOpType

    with tile.TileContext(nc) as tc:
        with tc.tile_pool(name="consts", bufs=1) as cp:
            xT_sb = cp.tile([P, N], dt.bfloat16)
            nc.sync.dma_start(out=xT_sb[:], in_=xT[:])
            xsT_sb = cp.tile([P, G * P], dt.bfloat16)
            nc.sync.dma_start(out=xsT_sb[:], in_=xsT[:])
            wl1_sb = cp.tile([F, D1], dt.bfloat16)
            nc.sync.dma_start(out=wl1_sb[:], in_=wl1[:])
            wr1_sb = cp.tile([F, D1], dt.bfloat16)
            nc.sync.dma_start(out=wr1_sb[:], in_=wr1[:])
            we1_sb = cp.tile([F, D1], dt.bfloat16)
            nc.sync.dma_start(out=we1_sb[:], in_=we1[:])
            att1_sb = cp.tile([P, D1], dt.bfloat16)
            nc.sync.dma_start(out=att1_sb[:], in_=att1r[:])
            wl2_sb = cp.tile([P, 16 * D2], dt.bfloat16)
            nc.sync.dma_start(out=wl2_sb[:], in_=wl2[:])
            wr2_sb = cp.tile([P, 16 * D2], dt.bfloat16)
            nc.sync.dma_start(out=wr2_sb[:], in_=wr2[:])
            we2_sb = cp.tile([F, D2], dt.bfloat16)
            nc.sync.dma_start(out=we2_sb[:], in_=we2[:])
            att2_sb = cp.tile([P, D2], dt.bfloat16)
            nc.sync.dma_start(out=att2_sb[:], in_=att2r[:])

            # ---------- phase A: node projections ----------
            if "A" in phases:
                with (
                  tc.tile_pool(name="a_ps", bufs=2, space="PSUM") as aps,
                  tc.tile_pool(name="a_sb", bufs=3) as asb,
              ):
                  # xl1 for ALL nodes (replicated compute)
                  for t in range((N + P - 1) // P):
                      mt = min(P, N - t * P)
                      ps = aps.tile([P, D1], dt.float32, tag="ps")
                      for j in range(4):
                          nc.tensor.matmul(
                              out=ps[:mt, j * 512:(j + 1) * 512],
                              lhsT=xT_sb[:, t * P:t * P + mt],
                              rhs=wl1_sb[:, j * 512:(j + 1) * 512],
                              start=True, stop=True,
                          )
                      xsb = asb.tile([P, D1], dt.bfloat16, tag="xsb")
                      nc.scalar.copy(out=xsb[:mt], in_=ps[:mt])
                      nc.sync.dma_start(out=xl1_tab[t * P:t * P + mt, :], in_=xsb[:mt])
                  # xr1 for own slice only
                  for g in range(G):
                      ps = aps.tile([P, D1], dt.float32, tag="ps")
                      for j in range(4):
                          nc.tensor.matmul(
                              out=ps[:GN, j * 512:(j + 1) * 512],
                              lhsT=xsT_sb[:, g * P:g * P + GN],
                              rhs=wr1_sb[:, j * 512:(j + 1) * 512],
                              start=True, stop=True,
                          )
                      xsb = asb.tile([P, D1], dt.bfloat16, tag="xsb")
                      nc.scalar.copy(out=xsb[:GN], in_=ps[:GN])
                      nc.sync.dma_start(out=xr1_sl[g * P:g * P + GN, :], in_=xsb[:GN])

            # ---------- phase B: layer-1 edge pass + layer-2 projections ----------
            if "B" in phases:
                with (
                  tc.tile_pool(name="b_acc", bufs=1, space="PSUM") as accp,   # 5 banks
                  tc.tile_pool(name="b_small", bufs=1, space="PSUM") as smallp,  # 1 bank
                  tc.tile_pool(name="b_eproj", bufs=1, space="PSUM") as eprojp,  # 2 banks
                  tc.tile_pool(name="b_sb", bufs=3) as bsb,
                  tc.tile_pool(name="b_sb2", bufs=2) as bsb2,
                  tc.tile_pool(name="saT", bufs=G) as satp,
              ):
                  saT_tiles = []
                  for g in range(G):
                      acc = accp.tile([P, D1 + 8], dt.float32, tag="acc")
                      selfsum = smallp.tile([P, F], dt.float32, tag="small")
                      invc_t = bsb.tile([P, 1], dt.float32, tag="invc")
                      nc.sync.dma_start(out=invc_t[:], in_=invc[g * P:(g + 1) * P, :])

                      for ch in range(CH):
                          is_self = ch == CH - 1
                          e0 = (g * CH + ch) * P
                          si = bsb.tile([P, 1], dt.int32, tag="si")
                          nc.sync.dma_start(out=si[:], in_=srci[e0:e0 + P, :])
                          di = bsb.tile([P, 1], dt.int32, tag="di")
                          nc.sync.dma_start(out=di[:], in_=dstpi[e0:e0 + P, :])
                          s01_t = bsb.tile([P, P], dt.bfloat16, tag="s01")
                          nc.sync.dma_start(out=s01_t[:], in_=s01[e0:e0 + P, :])
                          xl_g = bsb.tile([P, D1], dt.bfloat16, tag="xl")
                          nc.gpsimd.indirect_dma_start(
                              out=xl_g[:], out_offset=None, in_=xl1_tab[:],
                              in_offset=bass.IndirectOffsetOnAxis(ap=si[:, :1], axis=0))
                          xr_g = bsb.tile([P, D1], dt.bfloat16, tag="xr")
                          nc.gpsimd.indirect_dma_start(
                              out=xr_g[:], out_offset=None, in_=xr1_sl[:],
                              in_offset=bass.IndirectOffsetOnAxis(ap=di[:, :1], axis=0))

                          if not is_self:
                              eaT_t = bsb.tile([F, P], dt.bfloat16, tag="eaT")
                              nc.sync.dma_start(out=eaT_t[:], in_=eaT[:, e0:e0 + P])
                              eaN_t = bsb.tile([P, F], dt.bfloat16, tag="eaN")
                              nc.sync.dma_start(out=eaN_t[:], in_=eaN[e0:e0 + P, :])
                              nc.tensor.matmul(
                                  out=selfsum[:], lhsT=s01_t[:], rhs=eaN_t[:],
                                  start=(ch == 0), stop=(ch == CH - 2))
                              lhs_e = eaT_t
                          else:
                              sattr = bsb.tile([P, F], dt.bfloat16, tag="sattr")
                              nc.vector.tensor_scalar(
                                  out=sattr[:], in0=selfsum[:], scalar1=invc_t[:, :1],
                                  scalar2=None, op0=ALU.mult)
                              saT = satp.tile([P, F], dt.bfloat16, tag="saT")
                              nc.sync.dma_start(out=saT[:], in_=sattr[:], transpose=True)
                              saT_tiles.append(saT)
                              lhs_e = saT

                          logit = bsb.tile([P, H1], dt.float32, tag="logit")
                          for half in range(2):
                              c0 = half * 1024
                              ep = eprojp.tile([P, 1024], dt.float32, tag="eproj")
                              for j in range(2):
                                  nc.tensor.matmul(
                                      out=ep[:, j * 512:(j + 1) * 512],
                                      lhsT=lhs_e[:],
                                      rhs=we1_sb[:, c0 + j * 512:c0 + (j + 1) * 512],
                                      start=True, stop=True)
                              q = bsb2.tile([P, 1024], dt.bfloat16, tag="q")
                              nc.vector.tensor_add(
                                  out=q[:], in0=xl_g[:, c0:c0 + 1024],
                                  in1=xr_g[:, c0:c0 + 1024])
                              q2 = bsb2.tile([P, 1024], dt.bfloat16, tag="q2")
                              nc.vector.tensor_add(out=q2[:], in0=q[:], in1=ep[:])
                              a_t = bsb2.tile([P, 1024], dt.bfloat16, tag="a")
                              nc.scalar.activation(out=a_t[:], in_=q2[:], func=AF.Abs)
                              m_t = bsb2.tile([P, 1024], dt.bfloat16, tag="m")
                              nc.vector.scalar_tensor_tensor(
                                  out=m_t[:], in0=a_t[:], scalar=2.0 / 3.0,
                                  in1=q2[:], op0=ALU.mult, op1=ALU.add)
                              for hh in range(4):
                                  h = half * 4 + hh
                                  scr = bsb2.tile([P, C1], dt.bfloat16, tag="scr")
                                  nc.vector.scalar_tensor_tensor(
                                      out=scr[:],
                                      in0=m_t[:, hh * C1:(hh + 1) * C1],
                                      scalar=1.0,
                                      in1=att1_sb[:, h * C1:(h + 1) * C1],
                                      op0=ALU.mult, op1=ALU.mult,
                                      accum_out=logit[:, h:h + 1])
                          ex = bsb.tile([P, H1], dt.float32, tag="ex")
                          nc.scalar.activation(out=ex[:], in_=logit[:], func=AF.Exp)
                          xls = bsb.tile([P, D1 + 8], dt.bfloat16, tag="xls")
                          for h in range(H1):
                              nc.vector.tensor_scalar(
                                  out=xls[:, h * C1:(h + 1) * C1],
                                  in0=xl_g[:, h * C1:(h + 1) * C1],
                                  scalar1=ex[:, h:h + 1], scalar2=None, op0=ALU.mult)
                          nc.vector.tensor_copy(out=xls[:, D1:D1 + 8], in_=ex[:])
                          for j in range(4):
                              nc.tensor.matmul(
                                  out=acc[:, j * 512:(j + 1) * 512],
                                  lhsT=s01_t[:], rhs=xls[:, j * 512:(j + 1) * 512],
                                  start=(ch == 0), stop=(ch == CH - 1))
                          nc.tensor.matmul(
                              out=acc[:, D1:D1 + 8], lhsT=s01_t[:],
                              rhs=xls[:, D1:D1 + 8],
                              start=(ch == 0), stop=(ch == CH - 1))

                      # -- group finalize: h = relu(acc/denom), layer-2 projections
                      dn_r = bsb.tile([P, H1], dt.float32, tag="dnr")
                      nc.vector.reciprocal(out=dn_r[:], in_=acc[:, D1:D1 + 8])
                      h_sb = bsb.tile([P, D1], dt.bfloat16, tag="hg")
                      for h in range(H1):
                          nc.vector.tensor_scalar(
                              out=h_sb[:, h * C1:(h + 1) * C1],
                              in0=acc[:, h * C1:(h + 1) * C1],
                              scalar1=dn_r[:, h:h + 1], scalar2=0.0,
                              op0=ALU.mult, op1=ALU.max)
                      xl2_ps = smallp.tile([P, D2], dt.float32, tag="small")
                      xr2_ps = eprojp.tile([P, D2], dt.float32, tag="eproj")
                      for kk in range(16):
                          hT = bsb2.tile([P, P], dt.bfloat16, tag="hT")
                          nc.sync.dma_start(out=hT[:], in_=h_sb[:, kk * P:(kk + 1) * P],
                                            transpose=True)
                          nc.tensor.matmul(
                              out=xl2_ps[:], lhsT=hT[:],
                              rhs=wl2_sb[:, kk * D2:(kk + 1) * D2],
                              start=(kk == 0), stop=(kk == 15))
                          nc.tensor.matmul(
                              out=xr2_ps[:], lhsT=hT[:],
                              rhs=wr2_sb[:, kk * D2:(kk + 1) * D2],
                              start=(kk == 0), stop=(kk == 15))
                      xlr2_sb = bsb.tile([P, 2 * D2], dt.float32, tag="xlr2")
                      nc.vector.tensor_copy(out=xlr2_sb[:, :D2], in_=xl2_ps[:])
                      nc.vector.tensor_copy(out=xlr2_sb[:, D2:], in_=xr2_ps[:])
                      nc.sync.dma_start(out=xlr2[g * GN:(g + 1) * GN, :],
                                        in_=xlr2_sb[:GN])

            # ---------- AllGather of layer-2 projections ----------
            if "C" in phases:
              nc.gpsimd.collective_compute(
                  "AllGather", ALU.bypass, replica_groups=[list(range(M))],
                  ins=[xlr2[:]], outs=[ag_out[:]])
              nc.sync.dma_start(out=xl2_tab[:], in_=ag_out[:, :D2])
              nc.sync.dma_start(out=xr2_tab[:], in_=ag_out[:, D2:])

              # ---------- phase C: layer-2 edge pass ----------
              with (
                  tc.tile_pool(name="c_ps", bufs=1, space="PSUM") as cps,
                  tc.tile_pool(name="c_ps2", bufs=2, space="PSUM") as cps2,
                  tc.tile_pool(name="c_sb", bufs=3) as csb,
              ):
                  for g in range(G):
                      acc2 = cps.tile([P, D2 + 1], dt.float32, tag="acc2")
                      for ch in range(CH):
                          is_self = ch == CH - 1
                          e0 = (g * CH + ch) * P
                          si = csb.tile([P, 1], dt.int32, tag="si")
                          nc.sync.dma_start(out=si[:], in_=srci[e0:e0 + P, :])
                          dgi = csb.tile([P, 1], dt.int32, tag="dgi")
                          nc.sync.dma_start(out=dgi[:], in_=dstgi[e0:e0 + P, :])
                          s01_t = csb.tile([P, P], dt.bfloat16, tag="s01")
                          nc.sync.dma_start(out=s01_t[:], in_=s01[e0:e0 + P, :])
                          xl2e = csb.tile([P, D2], dt.float32, tag="xl2e")
                          nc.gpsimd.indirect_dma_start(
                              out=xl2e[:], out_offset=None, in_=xl2_tab[:],
                              in_offset=bass.IndirectOffsetOnAxis(ap=si[:, :1], axis=0))
                          xr2e = csb.tile([P, D2], dt.float32, tag="xr2e")
                          nc.gpsimd.indirect_dma_start(
                              out=xr2e[:], out_offset=None, in_=xr2_tab[:],
                              in_offset=bass.IndirectOffsetOnAxis(ap=dgi[:, :1], axis=0))
                          ep2 = cps2.tile([P, D2], dt.float32, tag="ep2")
                          if not is_self:
                              eaT_t = csb.tile([F, P], dt.bfloat16, tag="eaT")
                              nc.sync.dma_start(out=eaT_t[:], in_=eaT[:, e0:e0 + P])
                              lhs_e = eaT_t
                          else:
                              lhs_e = saT_tiles[g]
                          nc.tensor.matmul(out=ep2[:], lhsT=lhs_e[:], rhs=we2_sb[:],
                                           start=True, stop=True)
                          q2a = csb.tile([P, D2], dt.float32, tag="q2a")
                          nc.vector.tensor_add(out=q2a[:], in0=xl2e[:], in1=xr2e[:])
                          q2b = csb.tile([P, D2], dt.float32, tag="q2b")
                          nc.vector.tensor_add(out=q2b[:], in0=q2a[:], in1=ep2[:])
                          a2 = csb.tile([P, D2], dt.bfloat16, tag="a2")
                          nc.scalar.activation(out=a2[:], in_=q2b[:], func=AF.Abs)
                          m2 = csb.tile([P, D2], dt.bfloat16, tag="m2")
                          nc.vector.scalar_tensor_tensor(
                              out=m2[:], in0=a2[:], scalar=2.0 / 3.0,
                              in1=q2b[:], op0=ALU.mult, op1=ALU.add)
                          lgt2 = csb.tile([P, 1], dt.float32, tag="lgt2")
                          scr2 = csb.tile([P, D2], dt.bfloat16, tag="scr2")
                          nc.vector.scalar_tensor_tensor(
                              out=scr2[:], in0=m2[:], scalar=1.0, in1=att2_sb[:],
                              op0=ALU.mult, op1=ALU.mult,
                              accum_out=lgt2[:, :1])
                          ex2 = csb.tile([P, 1], dt.float32, tag="ex2")
                          nc.scalar.activation(out=ex2[:], in_=lgt2[:], func=AF.Exp)
                          xls2 = csb.tile([P, D2 + 1], dt.bfloat16, tag="xls2")
                          nc.vector.tensor_scalar(
                              out=xls2[:, :D2], in0=xl2e[:], scalar1=ex2[:, :1],
                              scalar2=None, op0=ALU.mult)
                          nc.vector.tensor_copy(out=xls2[:, D2:], in_=ex2[:])
                          nc.tensor.matmul(
                              out=acc2[:], lhsT=s01_t[:], rhs=xls2[:],
                              start=(ch == 0), stop=(ch == CH - 1))
                      d2r = csb.tile([P, 1], dt.float32, tag="d2r")
                      nc.vector.reciprocal(out=d2r[:], in_=acc2[:, D2:D2 + 1])
                      o2 = csb.tile([P, D2], dt.float32, tag="o2")
                      nc.vector.tensor_scalar(
                          out=o2[:], in0=acc2[:, :D2], scalar1=d2r[:, :1],
                          scalar2=0.0, op0=ALU.mult, op1=ALU.max)
                      nc.sync.dma_start(out=out[g * GN:(g + 1) * GN, :], in_=o2[:GN])

    nc.compile()
    return nc


def _prep_inputs(x, edge_index, edge_attr, Wl1, bl1, Wr1, br1, We1, att1, b1,
                 Wl2, bl2, Wr2, br2, We2, att2, b2):
    for b in (bl1, br1, b1, bl2, br2, b2):
        assert not np.any(np.asarray(b)), "nonzero biases not implemented"

    src = np.asarray(edge_index[0], dtype=np.int64)
    dst = np.asarray(edge_index[1], dtype=np.int64)
    ea = np.asarray(edge_attr, dtype=np.float32)
    order = np.argsort(dst, kind="stable")
    s_src, s_dst, s_ea = src[order], dst[order], ea[order]

    # group boundaries: 80 groups of GN dst nodes
    bounds = np.searchsorted(s_dst, np.arange(0, N + GN, GN))
    cnts = np.diff(bounds)  # edges per group (80,)
    G_CH = int(np.max((cnts + P - 1) // P))
    CH = G_CH + 1
    L = G * CH * P

    # per-node incoming counts
    node_cnt = np.bincount(s_dst, minlength=N).astype(np.float32)

    x = np.asarray(x, dtype=np.float32)
    common = {
        "xT": x.T.astype(BF16),
        "wl1": np.asarray(Wl1, np.float32).astype(BF16),
        "wr1": np.asarray(Wr1, np.float32).astype(BF16),
        "we1": np.asarray(We1, np.float32).astype(BF16),
        "att1r": (0.5 * (1 + NEG) * np.tile(
            np.asarray(att1, np.float32).reshape(1, D1), (P, 1))).astype(BF16),
        "wl2": np.asarray(Wl2, np.float32).reshape(16, P, D2)
               .transpose(1, 0, 2).reshape(P, 16 * D2).astype(BF16),
        "wr2": np.asarray(Wr2, np.float32).reshape(16, P, D2)
               .transpose(1, 0, 2).reshape(P, 16 * D2).astype(BF16),
        "we2": np.asarray(We2, np.float32).astype(BF16),
        "att2r": (0.5 * (1 + NEG) * np.tile(
            np.asarray(att2, np.float32).reshape(1, D2), (P, 1))).astype(BF16),
    }

    in_maps = []
    for k in range(M):
        base_node = k * NPC
        eaT_c = np.zeros((L, F), np.float32)   # will transpose at the end
        eaN_c = np.zeros((L, F), np.float32)
        s01_c = np.zeros((L, P), np.float32)
        srci_c = np.zeros((L, 1), np.int32)
        dstpi_c = np.zeros((L, 1), np.int32)
        dstgi_c = np.full((L, 1), base_node, np.int32)
        invc_c = np.ones((G * P, 1), np.float32)
        for g in range(G):
            gb = base_node + g * GN
            lo, hi = bounds[k * G + g], bounds[k * G + g + 1]
            cnt = hi - lo
            assert cnt <= G_CH * P
            o0 = (g * CH) * P  # stream offset of this group's first chunk
            # real edges
            eaT_c[o0:o0 + cnt] = s_ea[lo:hi]
            eaN_c[o0:o0 + cnt] = s_ea[lo:hi]
            dl = (s_dst[lo:hi] - gb).astype(np.int64)  # 0..GN-1
            s01_c[np.arange(o0, o0 + cnt), dl] = 1.0
            srci_c[o0:o0 + cnt, 0] = s_src[lo:hi]
            dstpi_c[o0:o0 + cnt, 0] = g * P + dl
            dstgi_c[o0:o0 + cnt, 0] = s_dst[lo:hi]
            # self chunk (last chunk of the group)
            so = (g * CH + CH - 1) * P
            nn = np.arange(GN)
            s01_c[so + nn, nn] = 1.0
            srci_c[so + nn, 0] = gb + nn
            dstpi_c[so + nn, 0] = g * P + nn
            dstgi_c[so + nn, 0] = gb + nn
            invc_c[g * P + nn, 0] = 1.0 / np.maximum(node_cnt[gb + nn], 1.0)
        im = dict(common)
        im["xsT"] = np.ascontiguousarray(
            np.pad(x[base_node:base_node + NPC].T.reshape(F, G, GN),
                   ((0, 0), (0, 0), (0, P - GN))).reshape(F, G * P)).astype(BF16)
        im["eaT"] = np.ascontiguousarray(eaT_c.T).astype(BF16)
        im["eaN"] = eaN_c.astype(BF16)
        im["s01"] = s01_c.astype(BF16)
        im["srci"] = srci_c
        im["dstpi"] = dstpi_c
        im["dstgi"] = dstgi_c
        im["invc"] = invc_c
        in_maps.append(im)
    return in_maps, CH


_PROG_CACHE = {}


def _get_program(CH, phases="ABC"):
    key = (CH, phases)
    if key not in _PROG_CACHE:
        _PROG_CACHE[key] = _build_program(CH, phases)
    return _PROG_CACHE[key]


def run(inputs, trace=False, tmpdir=None, phases="ABC"):
    in_maps, CH = _prep_inputs(**inputs)
    nc = _get_program(CH, phases)
    res = run_bass_kernel_spmd(nc, in_maps, list(range(M)), trace=trace,
                               tmpdir=tmpdir)
    outp = np.concatenate([res.results[k]["out"] for k in range(M)], axis=0)
    return outp.astype(np.float32), res


def kernel(**inputs):
    outp, _ = run(inputs)
    return outp



# revision 12
# speedup vs baseline: 1.3647x; 1.3647x over previous
"""GATv2 2-layer GNN on 8 Trainium2 NeuronCores (Bass/Tile, edge-parallel).

Sharding: edges sorted by dst node, dst-range sharded across 8 cores
(core k owns dst nodes [1250k, 1250(k+1))), so the per-dst segment
softmax and aggregation are fully core-local. Self-loop edge attrs
(mean of incoming) are precomputed host-side and folded into the edge
stream. Layer-1 node projections are computed replicated (xl1 for all
nodes; xr1 for the own slice). Between layers only the 32-wide xl2
projection is exchanged with a single AllGather (xr2 is dst-local).

Per-edge-chunk pipeline (128 edges):
  gpsimd: q = xl1[src] (+)DMA xr1[dst]          (fused add-gather, bf16)
  tensor: ep = ea @ We1 into PSUM (2x 1024-wide matmuls)
  vector: z = q + ep ; 8x logit-accum STT ; 8x xls = q*ex
  scalar: m = LeakyRelu(z) ; ex = Exp(logits)
  tensor: acc += s01^T @ xls ; den += s01^T @ ex
Aggregation identity: sum_e alpha*(xl+xr) = out[d] + xr[d] (softmax
sums to 1), so the group finalize computes h = relu(acc/den - xr[d])
and xl never needs to be gathered standalone.
"""
import sys
sys.path.insert(0, "/opt/trn_rl_repo")

import numpy as np
import ml_dtypes

import concourse.bass as bass
import concourse.bacc as bacc
import concourse.tile as tile
from concourse import mybir
from concourse.bass_utils import run_bass_kernel_spmd

BF16 = ml_dtypes.bfloat16

N, E, F = 10000, 80000, 128
H1, C1 = 8, 256
D1 = H1 * C1          # 2048
D2 = 32               # layer-2 out (1 head)
NEG = 0.2
M = 8                 # cores
NPC = N // M          # 1250 nodes per core
GN = 125              # dst nodes per group
G = NPC // GN         # 10 groups per core
P = 128

dt = mybir.dt


def _build_program(CH):
    """Build the SPMD Bass program. CH = chunks per group (self edges incl)."""
    NCH = G * CH          # chunks per core
    L = NCH * P           # edge-stream length per core
    nc = bacc.Bacc("TRN2", target_bir_lowering=False, debug=False, num_devices=M)

    ei = {}
    def EIN(name, shape, dtype):
        ei[name] = nc.dram_tensor(name, list(shape), dtype, kind="ExternalInput")
        return ei[name]

    xT    = EIN("xT",    (P, N),       dt.bfloat16)   # x transposed (replicated)
    xsT   = EIN("xsT",   (P, G * P),   dt.bfloat16)   # own-slice cols, group padded
    wl1   = EIN("wl1",   (F, D1),      dt.bfloat16)
    wr1   = EIN("wr1",   (F, D1),      dt.bfloat16)
    we1   = EIN("we1",   (F, D1),      dt.bfloat16)
    att1r = EIN("att1r", (P, D1),      dt.bfloat16)   # att1 row-replicated (raw)
    wlr2  = EIN("wlr2",  (P, 16 * 2 * D2), dt.bfloat16)  # [p, k*64+c]: Wl2|Wr2
    we2   = EIN("we2",   (F, D2),      dt.bfloat16)
    att2r = EIN("att2r", (P, D2),      dt.bfloat16)
    ident = EIN("ident", (P, P),       dt.bfloat16)   # identity (PE transpose)
    eaT   = EIN("eaT",   (F, L),       dt.bfloat16)   # edge_attr^T, sorted+padded
    s01T  = EIN("s01T",  (P, L),       dt.bfloat16)   # [p, ch*128+d]: edge p of
                                                      # chunk ch one-hot on dst d
    srci  = EIN("srci",  (P, NCH),     dt.int32)      # global src id per slot
    dstpi = EIN("dstpi", (P, NCH),     dt.int32)      # group-padded local dst id

    out = nc.dram_tensor("out", [NPC, D2], dt.float32, kind="ExternalOutput")
    dbg_h = nc.dram_tensor("dbg_h", [G * P, D1], dt.bfloat16,
                           kind="ExternalOutput")
    dbg_q = nc.dram_tensor("dbg_q", [P, D1], dt.bfloat16, kind="ExternalOutput")
    dbg_lg = nc.dram_tensor("dbg_lg", [P, 8], dt.float32, kind="ExternalOutput")
    dbg_xl2 = nc.dram_tensor("dbg_xl2", [NPC, D2], dt.float32,
                             kind="ExternalOutput")
    dbg_xr2 = nc.dram_tensor("dbg_xr2", [G * P, D2], dt.float32,
                             kind="ExternalOutput")
    dbg_q2 = nc.dram_tensor("dbg_q2", [P, D2], dt.float32,
                            kind="ExternalOutput")
    dbg_lg2 = nc.dram_tensor("dbg_lg2", [P, 1], dt.float32,
                             kind="ExternalOutput")

    # DRAM scratch
    xl1_tab = nc.dram_tensor("xl1_tab", [N, D1], dt.bfloat16)
    xr1_sl  = nc.dram_tensor("xr1_sl", [G * P, D1], dt.bfloat16)
    xl2_own = nc.dram_tensor("xl2_own", [NPC, D2], dt.float32)
    xr2_tab = nc.dram_tensor("xr2_tab", [G * P, D2], dt.float32)
    xl2_all = nc.dram_tensor("xl2_all", [N, D2], dt.float32, addr_space="Shared")
    dum0    = nc.dram_tensor("dum0", [1, 8], dt.bfloat16)
    dumAG   = nc.dram_tensor("dumAG", [M, 8], dt.bfloat16, addr_space="Shared")
    dumsink = nc.dram_tensor("dumsink", [1, 8], dt.bfloat16)

    AF = mybir.ActivationFunctionType
    ALU = mybir.AluOpType
    RG = [list(range(M))]

    with tile.TileContext(nc) as tc:
        with tc.tile_pool(name="consts", bufs=1) as cp:
            def CONST(name, src, shape, dtype):
                t = cp.tile(list(shape), dtype, name=name)
                nc.sync.dma_start(out=t[:], in_=src[:])
                return t

            xT_sb   = CONST("xT_sb", xT, (P, N), dt.bfloat16)
            xsT_sb  = CONST("xsT_sb", xsT, (P, G * P), dt.bfloat16)
            wl1_sb  = CONST("wl1_sb", wl1, (F, D1), dt.bfloat16)
            wr1_sb  = CONST("wr1_sb", wr1, (F, D1), dt.bfloat16)
            we1_sb  = CONST("we1_sb", we1, (F, D1), dt.bfloat16)
            att1_sb = CONST("att1_sb", att1r, (P, D1), dt.bfloat16)
            wlr2_sb = CONST("wlr2_sb", wlr2, (P, 32 * D2), dt.bfloat16)
            we2_sb  = CONST("we2_sb", we2, (F, D2), dt.bfloat16)
            att2_sb = CONST("att2_sb", att2r, (P, D2), dt.bfloat16)
            id_sb   = CONST("id_sb", ident, (P, P), dt.bfloat16)
            eaT_sb  = CONST("eaT_sb", eaT, (F, L), dt.bfloat16)
            s01_sb  = CONST("s01_sb", s01T, (P, L), dt.bfloat16)
            srci_sb = CONST("srci_sb", srci, (P, NCH), dt.int32)
            dstpi_sb = CONST("dstpi_sb", dstpi, (P, NCH), dt.int32)
            ep2_sb = cp.tile([P, NCH * D2], dt.bfloat16, name="ep2_sb")

            # warm-up collective: absorbs CC cold-start under phase A/B
            nc.sync.dma_start(out=dum0[:], in_=xT[0:1, 0:8])
            nc.gpsimd.collective_compute(
                "AllGather", ALU.bypass, replica_groups=RG,
                ins=[dum0[:]], outs=[dumAG[:]])
            nc.sync.dma_start(out=dumsink[:], in_=dumAG[0:1])

            # ---------- phase A: node projections + ep2 precompute ----------
            with (
                tc.tile_pool(name="a_ps", bufs=3, space="PSUM") as aps,
                tc.tile_pool(name="a_ps2", bufs=1, space="PSUM") as aps2,
                tc.tile_pool(name="a_sb", bufs=4) as asb,
            ):
                NT = (N + P - 1) // P
                for t in range(NT + G):
                    if t < NT:  # xl1 for ALL nodes (replicated compute)
                        mt = min(P, N - t * P)
                        lhs = xT_sb[:, t * P:t * P + mt]
                        w = wl1_sb
                        dst_tab, r0 = xl1_tab, t * P
                    else:       # xr1 for own slice
                        g = t - NT
                        mt = GN
                        lhs = xsT_sb[:, g * P:g * P + mt]
                        w = wr1_sb
                        dst_tab, r0 = xr1_sl, g * P
                    for half in range(2):
                        ps = aps.tile([P, 1024], dt.float32, tag="ps")
                        for j in range(2):
                            nc.tensor.matmul(
                                out=ps[:mt, j * 512:(j + 1) * 512],
                                lhsT=lhs,
                                rhs=w[:, half * 1024 + j * 512:
                                      half * 1024 + (j + 1) * 512],
                                start=True, stop=True)
                        xsb = asb.tile([P, 1024], dt.bfloat16, tag="xsb")
                        if half == 0:
                            nc.scalar.copy(out=xsb[:mt], in_=ps[:mt])
                        else:
                            nc.vector.tensor_copy(out=xsb[:mt], in_=ps[:mt])
                        nc.sync.dma_start(
                            out=dst_tab[r0:r0 + mt,
                                        half * 1024:(half + 1) * 1024],
                            in_=xsb[:mt])
                # layer-2 edge projections for all chunks (resident)
                NB = (NCH + 15) // 16
                for blk in range(NB):
                    ps2 = aps2.tile([P, 512], dt.float32, tag="ep2")
                    n_in_blk = min(16, NCH - blk * 16)
                    for j in range(n_in_blk):
                        ch = blk * 16 + j
                        nc.tensor.matmul(
                            out=ps2[:, j * D2:(j + 1) * D2],
                            lhsT=eaT_sb[:, ch * P:(ch + 1) * P],
                            rhs=we2_sb[:], start=True, stop=True)
                    nc.vector.tensor_copy(
                        out=ep2_sb[:, blk * 512:blk * 512 + n_in_blk * D2],
                        in_=ps2[:, :n_in_blk * D2])

            # ---------- phase B: layer-1 edge pass ----------
            xr2_tiles = []
            with tc.tile_pool(name="xr2res", bufs=G) as xr2p:
              with (
                tc.tile_pool(name="b_acc", bufs=1, space="PSUM") as accp,  # 4
                tc.tile_pool(name="b_ep", bufs=1, space="PSUM") as epp,    # 2
                tc.tile_pool(name="b_sm", bufs=1, space="PSUM") as smp,    # 1
                tc.tile_pool(name="b_x2", bufs=1, space="PSUM") as x2p,    # 1
                tc.tile_pool(name="b_q", bufs=4) as qp,
                tc.tile_pool(name="b_z", bufs=3) as zp,
                tc.tile_pool(name="b_m", bufs=3) as mp,
                tc.tile_pool(name="b_xls", bufs=3) as xlsp,
                tc.tile_pool(name="b_sc", bufs=4) as scp,
                tc.tile_pool(name="b_fin", bufs=2) as finp,
              ):
                for g in range(G):
                    acc = accp.tile([P, D1], dt.float32, tag="acc")
                    den = smp.tile([P, 8], dt.float32, tag="sm")
                    for ch in range(CH):
                        chb = g * CH + ch
                        e0 = chb * P
                        q = qp.tile([P, D1], dt.bfloat16, tag="q")
                        nc.gpsimd.indirect_dma_start(
                            out=q[:], out_offset=None, in_=xl1_tab[:],
                            in_offset=bass.IndirectOffsetOnAxis(
                                ap=srci_sb[:, chb:chb + 1], axis=0))
                        nc.gpsimd.indirect_dma_start(
                            out=q[:], out_offset=None, in_=xr1_sl[:],
                            in_offset=bass.IndirectOffsetOnAxis(
                                ap=dstpi_sb[:, chb:chb + 1], axis=0),
                            compute_op=ALU.add)
                        z = zp.tile([P, D1], dt.bfloat16, tag="z")
                        m = mp.tile([P, D1], dt.bfloat16, tag="m")
                        logit = scp.tile([P, 8], dt.float32, tag="lg")
                        ex = scp.tile([P, 8], dt.float32, tag="ex")
                        exb = scp.tile([P, 8], dt.bfloat16, tag="exb")
                        xls = xlsp.tile([P, D1], dt.bfloat16, tag="xls")
                        for half in range(2):
                            c0 = half * 1024
                            ep = epp.tile([P, 1024], dt.float32, tag="ep")
                            for j in range(2):
                                nc.tensor.matmul(
                                    out=ep[:, j * 512:(j + 1) * 512],
                                    lhsT=eaT_sb[:, e0:e0 + P],
                                    rhs=we1_sb[:, c0 + j * 512:
                                               c0 + (j + 1) * 512],
                                    start=True, stop=True)
                            nc.vector.tensor_add(
                                out=z[:, c0:c0 + 1024],
                                in0=q[:, c0:c0 + 1024], in1=ep[:])
                            nc.scalar.activation(
                                out=m[:, c0:c0 + 1024],
                                in_=z[:, c0:c0 + 1024],
                                func=AF.Prelu, alpha=NEG)
                        for h in range(H1):
                            nc.vector.scalar_tensor_tensor(
                                out=z[:, h * C1:(h + 1) * C1],
                                in0=m[:, h * C1:(h + 1) * C1], scalar=1.0,
                                in1=att1_sb[:, h * C1:(h + 1) * C1],
                                op0=ALU.mult, op1=ALU.mult,
                                accum_out=logit[:, h:h + 1])
                        if g == 0 and ch == 0:
                            nc.sync.dma_start(out=dbg_q[:], in_=q[:])
                            nc.sync.dma_start(out=dbg_lg[:], in_=logit[:])
                        nc.scalar.activation(out=ex[:], in_=logit[:],
                                             func=AF.Exp)
                        nc.scalar.copy(out=exb[:], in_=ex[:])
                        for h in range(H1):
                            nc.vector.tensor_scalar(
                                out=xls[:, h * C1:(h + 1) * C1],
                                in0=q[:, h * C1:(h + 1) * C1],
                                scalar1=ex[:, h:h + 1], scalar2=None,
                                op0=ALU.mult)
                        for j in range(4):
                            nc.tensor.matmul(
                                out=acc[:, j * 512:(j + 1) * 512],
                                lhsT=s01_sb[:, e0:e0 + P],
                                rhs=xls[:, j * 512:(j + 1) * 512],
                                start=(ch == 0), stop=(ch == CH - 1))
                        nc.tensor.matmul(
                            out=den[:], lhsT=s01_sb[:, e0:e0 + P], rhs=exb[:],
                            start=(ch == 0), stop=(ch == CH - 1))

                    # ---- group finalize ----
                    dn = scp.tile([P, 8], dt.float32, tag="dn")
                    nc.vector.reciprocal(out=dn[:], in_=den[:])
                    xr_g = finp.tile([P, D1], dt.bfloat16, tag="xrg")
                    nc.sync.dma_start(out=xr_g[:],
                                      in_=xr1_sl[g * P:(g + 1) * P, :])
                    hs = finp.tile([P, D1], dt.bfloat16, tag="hs")
                    for h in range(H1):
                        nc.vector.scalar_tensor_tensor(
                            out=hs[:, h * C1:(h + 1) * C1],
                            in0=acc[:, h * C1:(h + 1) * C1],
                            scalar=dn[:, h:h + 1],
                            in1=xr_g[:, h * C1:(h + 1) * C1],
                            op0=ALU.mult, op1=ALU.subtract)
                    hr = finp.tile([P, D1], dt.bfloat16, tag="hr")
                    nc.scalar.activation(out=hr[:], in_=hs[:], func=AF.Relu)
                    nc.sync.dma_start(out=dbg_h[g * P:(g + 1) * P, :],
                                      in_=hr[:])
                    x2ps = x2p.tile([P, 2 * D2], dt.float32, tag="x2")
                    for half in range(2):
                        tp = smp.tile([P, 1024], dt.bfloat16, tag="sm")
                        for k in range(8):
                            kk = half * 8 + k
                            nc.tensor.transpose(
                                out=tp[:, k * P:(k + 1) * P],
                                in_=hr[:, kk * P:(kk + 1) * P],
                                identity=id_sb[:])
                        hT = finp.tile([P, 1024], dt.bfloat16, tag="hT")
                        nc.vector.tensor_copy(out=hT[:], in_=tp[:])
                        for k in range(8):
                            kk = half * 8 + k
                            nc.tensor.matmul(
                                out=x2ps[:, 0:2 * D2],
                                lhsT=hT[:, k * P:(k + 1) * P],
                                rhs=wlr2_sb[:, kk * 2 * D2:(kk + 1) * 2 * D2],
                                start=(kk == 0), stop=(kk == 15))
                    x2sb = finp.tile([P, 2 * D2], dt.float32, tag="x2sb")
                    nc.vector.tensor_copy(out=x2sb[:], in_=x2ps[:])
                    nc.sync.dma_start(out=xl2_own[g * GN:(g + 1) * GN, :],
                                      in_=x2sb[:GN, 0:D2])
                    nc.sync.dma_start(out=xr2_tab[g * P:(g + 1) * P, :],
                                      in_=x2sb[:, D2:2 * D2])
                    xr2_res = xr2p.tile([P, D2], dt.float32, tag="xr2")
                    nc.vector.tensor_copy(out=xr2_res[:], in_=x2sb[:, D2:])
                    xr2_tiles.append(xr2_res)

              nc.sync.dma_start(out=dbg_xl2[:], in_=xl2_own[:])
              nc.sync.dma_start(out=dbg_xr2[:], in_=xr2_tab[:])
              # ---------- AllGather of xl2 ----------
              if True:
                nc.gpsimd.collective_compute(
                    "AllGather", ALU.bypass, replica_groups=RG,
                    ins=[xl2_own[:]], outs=[xl2_all[:]])

                # ---------- phase C: layer-2 edge pass ----------
                with (
                    tc.tile_pool(name="c_ps", bufs=2, space="PSUM") as cps,
                    tc.tile_pool(name="c_q", bufs=NCH) as cqp,
                    tc.tile_pool(name="c_sb", bufs=4) as csb,
                ):
                    # prefetch local xr2[dst] for every chunk (no AG dep)
                    q2_tiles = []
                    for chb in range(NCH):
                        q2 = cqp.tile([P, D2], dt.float32, tag="q2")
                        nc.gpsimd.indirect_dma_start(
                            out=q2[:], out_offset=None, in_=xr2_tab[:],
                            in_offset=bass.IndirectOffsetOnAxis(
                                ap=dstpi_sb[:, chb:chb + 1], axis=0))
                        q2_tiles.append(q2)
                    for g in range(G):
                        acc2 = cps.tile([P, D2 + 1], dt.float32, tag="a2")
                        for ch in range(CH):
                            chb = g * CH + ch
                            e0 = chb * P
                            q2 = q2_tiles[chb]
                            nc.gpsimd.indirect_dma_start(
                                out=q2[:], out_offset=None, in_=xl2_all[:],
                                in_offset=bass.IndirectOffsetOnAxis(
                                    ap=srci_sb[:, chb:chb + 1], axis=0),
                                compute_op=ALU.add)
                            z2 = csb.tile([P, D2], dt.float32, tag="z2")
                            nc.vector.tensor_add(
                                out=z2[:], in0=q2[:],
                                in1=ep2_sb[:, chb * D2:(chb + 1) * D2])
                            m2 = csb.tile([P, D2], dt.bfloat16, tag="m2")
                            nc.scalar.activation(out=m2[:], in_=z2[:],
                                                 func=AF.Prelu, alpha=NEG)
                            lg2 = csb.tile([P, 1], dt.float32, tag="lg2")
                            sc2 = csb.tile([P, D2], dt.bfloat16, tag="sc2")
                            nc.vector.scalar_tensor_tensor(
                                out=sc2[:], in0=m2[:], scalar=1.0,
                                in1=att2_sb[:], op0=ALU.mult, op1=ALU.mult,
                                accum_out=lg2[:, :1])
                            if g == 0 and ch == 0:
                                nc.sync.dma_start(out=dbg_q2[:], in_=q2[:])
                                nc.sync.dma_start(out=dbg_lg2[:], in_=lg2[:])
                            xls2 = csb.tile([P, D2 + 1], dt.bfloat16,
                                            tag="xls2")
                            ex2 = csb.tile([P, 1], dt.float32, tag="ex2")
                            nc.scalar.activation(out=ex2[:], in_=lg2[:],
                                                 func=AF.Exp)
                            nc.scalar.copy(out=xls2[:, D2:D2 + 1], in_=ex2[:])
                            nc.vector.tensor_scalar(
                                out=xls2[:, :D2], in0=q2[:],
                                scalar1=ex2[:, :1], scalar2=None,
                                op0=ALU.mult)
                            nc.tensor.matmul(
                                out=acc2[:], lhsT=s01_sb[:, e0:e0 + P],
                                rhs=xls2[:],
                                start=(ch == 0), stop=(ch == CH - 1))
                        d2 = csb.tile([P, 1], dt.float32, tag="d2")
                        nc.vector.reciprocal(out=d2[:], in_=acc2[:, D2:D2 + 1])
                        o2 = csb.tile([P, D2], dt.float32, tag="o2")
                        nc.vector.scalar_tensor_tensor(
                            out=o2[:], in0=acc2[:, :D2], scalar=d2[:, :1],
                            in1=xr2_tiles[g], op0=ALU.mult, op1=ALU.subtract)
                        orl = csb.tile([P, D2], dt.float32, tag="orl")
                        nc.vector.tensor_scalar(
                            out=orl[:], in0=o2[:], scalar1=0.0, scalar2=None,
                            op0=ALU.max)
                        nc.sync.dma_start(out=out[g * GN:(g + 1) * GN, :],
                                          in_=orl[:GN])

    nc.compile()
    return nc


def _prep_inputs(x, edge_index, edge_attr, Wl1, bl1, Wr1, br1, We1, att1, b1,
                 Wl2, bl2, Wr2, br2, We2, att2, b2):
    for b in (bl1, br1, b1, bl2, br2, b2):
        assert not np.any(np.asarray(b)), "nonzero biases not implemented"

    src = np.asarray(edge_index[0], dtype=np.int64)
    dst = np.asarray(edge_index[1], dtype=np.int64)
    ea = np.asarray(edge_attr, dtype=np.float32)

    # PyG fill_value='mean' self loops, computed host-side
    cnt = np.bincount(dst, minlength=N).astype(np.float32)
    ssum = np.zeros((N, F), np.float32)
    np.add.at(ssum, dst, ea)
    self_attr = ssum / np.maximum(cnt, 1.0)[:, None]

    order = np.argsort(dst, kind="stable")
    s_src, s_dst, s_ea = src[order], dst[order], ea[order]
    bounds = np.searchsorted(s_dst, np.arange(0, N + GN, GN))
    cnts = np.diff(bounds)                       # real edges per group (80,)
    CH = int(np.max((cnts + GN + P - 1) // P))   # incl. GN self edges
    NCH = G * CH
    L = NCH * P

    x = np.asarray(x, dtype=np.float32)
    common = {
        "xT": x.T.astype(BF16),
        "wl1": np.asarray(Wl1, np.float32).astype(BF16),
        "wr1": np.asarray(Wr1, np.float32).astype(BF16),
        "we1": np.asarray(We1, np.float32).astype(BF16),
        "att1r": np.tile(np.asarray(att1, np.float32).reshape(1, D1),
                         (P, 1)).astype(BF16),
        "wlr2": np.concatenate([
                    np.asarray(Wl2, np.float32).reshape(16, P, D2),
                    np.asarray(Wr2, np.float32).reshape(16, P, D2)],
                    axis=2).transpose(1, 0, 2).reshape(P, 32 * D2)
                .astype(BF16),
        "we2": np.asarray(We2, np.float32).astype(BF16),
        "att2r": np.tile(np.asarray(att2, np.float32).reshape(1, D2),
                         (P, 1)).astype(BF16),
        "ident": np.eye(P, dtype=np.float32).astype(BF16),
    }

    in_maps = []
    for k in range(M):
        base_node = k * NPC
        ea_c = np.zeros((L, F), np.float32)
        s01_c = np.zeros((L, P), np.float32)
        srci_c = np.zeros((L,), np.int32)
        dstpi_c = np.zeros((L,), np.int32)
        for g in range(G):
            gb = base_node + g * GN
            lo, hi = bounds[k * G + g], bounds[k * G + g + 1]
            cnt_g = hi - lo
            tot = cnt_g + GN
            assert tot <= CH * P
            o0 = g * CH * P
            sl = np.arange(o0, o0 + tot)
            ea_c[sl[:cnt_g]] = s_ea[lo:hi]
            ea_c[sl[cnt_g:]] = self_attr[gb:gb + GN]
            dl = np.concatenate([(s_dst[lo:hi] - gb), np.arange(GN)])
            s01_c[sl, dl] = 1.0
            srci_c[sl] = np.concatenate([s_src[lo:hi], np.arange(gb, gb + GN)])
            dstpi_c[sl] = g * P + dl
        im = dict(common)
        im["xsT"] = np.ascontiguousarray(
            np.pad(x[base_node:base_node + NPC].T.reshape(F, G, GN),
                   ((0, 0), (0, 0), (0, P - GN))).reshape(F, G * P)).astype(BF16)
        im["eaT"] = np.ascontiguousarray(ea_c.T).astype(BF16)
        # [p, ch*128+d] layout: edge slot p of chunk ch
        im["s01T"] = np.ascontiguousarray(
            s01_c.reshape(NCH, P, P).transpose(1, 0, 2)
            .reshape(P, L)).astype(BF16)
        im["srci"] = np.ascontiguousarray(srci_c.reshape(NCH, P).T)
        im["dstpi"] = np.ascontiguousarray(dstpi_c.reshape(NCH, P).T)
        in_maps.append(im)
    return in_maps, CH


_PROG_CACHE = {}


def _get_program(CH):
    if CH not in _PROG_CACHE:
        _PROG_CACHE[CH] = _build_program(CH)
    return _PROG_CACHE[CH]


def run(inputs, trace=False, tmpdir=None):
    in_maps, CH = _prep_inputs(**inputs)
    nc = _get_program(CH)
    res = run_bass_kernel_spmd(nc, in_maps, list(range(M)), trace=trace,
                               tmpdir=tmpdir)
    outp = np.concatenate([res.results[k]["out"] for k in range(M)], axis=0)
    return outp.astype(np.float32), res


def kernel(**inputs):
    outp, _ = run(inputs)
    return outp


# revision 13
# speedup vs baseline: 1.3681x; 1.0025x over previous
"""GATv2 2-layer GNN on 8 Trainium2 NeuronCores (Bass/Tile, edge-parallel).

Sharding: edges sorted by dst node, dst-range sharded across 8 cores
(core k owns dst nodes [1250k, 1250(k+1))), so the per-dst segment
softmax and aggregation are fully core-local. Self-loop edge attrs
(mean of incoming) are precomputed host-side and folded into the edge
stream. Layer-1 node projections are computed replicated (xl1 for all
nodes; xr1 for the own slice). Between layers only the 32-wide xl2
projection is exchanged with a single AllGather (xr2 is dst-local).

Per-edge-chunk pipeline (128 edges):
  gpsimd: q = xl1[src] (+)DMA xr1[dst]          (fused add-gather, bf16)
  tensor: ep = ea @ We1 into PSUM (2x 1024-wide matmuls)
  vector: z = q + ep ; 8x logit-accum STT ; 8x xls = q*ex
  scalar: m = LeakyRelu(z) ; ex = Exp(logits)
  tensor: acc += s01^T @ xls ; den += s01^T @ ex
Aggregation identity: sum_e alpha*(xl+xr) = out[d] + xr[d] (softmax
sums to 1), so the group finalize computes h = relu(acc/den - xr[d])
and xl never needs to be gathered standalone.
"""
import sys
sys.path.insert(0, "/opt/trn_rl_repo")

import numpy as np
import ml_dtypes

import concourse.bass as bass
import concourse.bacc as bacc
import concourse.tile as tile
from concourse import mybir
from concourse.bass_utils import run_bass_kernel_spmd

BF16 = ml_dtypes.bfloat16

N, E, F = 10000, 80000, 128
H1, C1 = 8, 256
D1 = H1 * C1          # 2048
D2 = 32               # layer-2 out (1 head)
NEG = 0.2
M = 8                 # cores
NPC = N // M          # 1250 nodes per core
GN = 125              # dst nodes per group
G = NPC // GN         # 10 groups per core
P = 128

dt = mybir.dt


def _build_program(CH):
    """Build the SPMD Bass program. CH = chunks per group (self edges incl)."""
    NCH = G * CH          # chunks per core
    L = NCH * P           # edge-stream length per core
    nc = bacc.Bacc("TRN2", target_bir_lowering=False, debug=False, num_devices=M)

    ei = {}
    def EIN(name, shape, dtype):
        ei[name] = nc.dram_tensor(name, list(shape), dtype, kind="ExternalInput")
        return ei[name]

    xT    = EIN("xT",    (P, N),       dt.bfloat16)   # x transposed (replicated)
    xsT   = EIN("xsT",   (P, G * P),   dt.bfloat16)   # own-slice cols, group padded
    wl1   = EIN("wl1",   (F, D1),      dt.bfloat16)
    wr1   = EIN("wr1",   (F, D1),      dt.bfloat16)
    we1   = EIN("we1",   (F, D1),      dt.bfloat16)
    att1r = EIN("att1r", (P, D1),      dt.bfloat16)   # att1 row-replicated (raw)
    wlr2  = EIN("wlr2",  (P, 16 * 2 * D2), dt.bfloat16)  # [p, k*64+c]: Wl2|Wr2
    we2   = EIN("we2",   (F, D2),      dt.bfloat16)
    att2r = EIN("att2r", (P, D2),      dt.bfloat16)
    ident = EIN("ident", (P, P),       dt.bfloat16)   # identity (PE transpose)
    eaT   = EIN("eaT",   (F, L),       dt.bfloat16)   # edge_attr^T, sorted+padded
    s01T  = EIN("s01T",  (P, L),       dt.bfloat16)   # [p, ch*128+d]: edge p of
                                                      # chunk ch one-hot on dst d
    srci  = EIN("srci",  (P, NCH),     dt.int32)      # global src id per slot
    dstpi = EIN("dstpi", (P, NCH),     dt.int32)      # group-padded local dst id

    out = nc.dram_tensor("out", [NPC, D2], dt.float32, kind="ExternalOutput")

    # DRAM scratch
    xl1_tab = nc.dram_tensor("xl1_tab", [N, D1], dt.bfloat16)
    xr1_sl  = nc.dram_tensor("xr1_sl", [G * P, D1], dt.bfloat16)
    xl2_own = nc.dram_tensor("xl2_own", [NPC, D2], dt.float32)
    xr2_tab = nc.dram_tensor("xr2_tab", [G * P, D2], dt.float32)
    xl2_all = nc.dram_tensor("xl2_all", [N, D2], dt.float32, addr_space="Shared")
    dum0    = nc.dram_tensor("dum0", [1, 8], dt.bfloat16)
    dumAG   = nc.dram_tensor("dumAG", [M, 8], dt.bfloat16, addr_space="Shared")
    dumsink = nc.dram_tensor("dumsink", [1, 8], dt.bfloat16)

    AF = mybir.ActivationFunctionType
    ALU = mybir.AluOpType
    RG = [list(range(M))]

    with tile.TileContext(nc) as tc:
        with tc.tile_pool(name="consts", bufs=1) as cp:
            def CONST(name, src, shape, dtype):
                t = cp.tile(list(shape), dtype, name=name)
                nc.sync.dma_start(out=t[:], in_=src[:])
                return t

            xT_sb   = CONST("xT_sb", xT, (P, N), dt.bfloat16)
            xsT_sb  = CONST("xsT_sb", xsT, (P, G * P), dt.bfloat16)
            wl1_sb  = CONST("wl1_sb", wl1, (F, D1), dt.bfloat16)
            wr1_sb  = CONST("wr1_sb", wr1, (F, D1), dt.bfloat16)
            we1_sb  = CONST("we1_sb", we1, (F, D1), dt.bfloat16)
            att1_sb = CONST("att1_sb", att1r, (P, D1), dt.bfloat16)
            wlr2_sb = CONST("wlr2_sb", wlr2, (P, 32 * D2), dt.bfloat16)
            we2_sb  = CONST("we2_sb", we2, (F, D2), dt.bfloat16)
            att2_sb = CONST("att2_sb", att2r, (P, D2), dt.bfloat16)
            id_sb   = CONST("id_sb", ident, (P, P), dt.bfloat16)
            eaT_sb  = CONST("eaT_sb", eaT, (F, L), dt.bfloat16)
            s01_sb  = CONST("s01_sb", s01T, (P, L), dt.bfloat16)
            srci_sb = CONST("srci_sb", srci, (P, NCH), dt.int32)
            dstpi_sb = CONST("dstpi_sb", dstpi, (P, NCH), dt.int32)
            ep2_sb = cp.tile([P, NCH * D2], dt.bfloat16, name="ep2_sb")

            # warm-up collective: absorbs CC cold-start under phase A/B
            nc.sync.dma_start(out=dum0[:], in_=xT[0:1, 0:8])
            nc.gpsimd.collective_compute(
                "AllGather", ALU.bypass, replica_groups=RG,
                ins=[dum0[:]], outs=[dumAG[:]])
            nc.sync.dma_start(out=dumsink[:], in_=dumAG[0:1])

            # ---------- phase A: node projections + ep2 precompute ----------
            with (
                tc.tile_pool(name="a_ps", bufs=3, space="PSUM") as aps,
                tc.tile_pool(name="a_ps2", bufs=1, space="PSUM") as aps2,
                tc.tile_pool(name="a_sb", bufs=4) as asb,
            ):
                NT = (N + P - 1) // P
                for t in range(NT + G):
                    if t < NT:  # xl1 for ALL nodes (replicated compute)
                        mt = min(P, N - t * P)
                        lhs = xT_sb[:, t * P:t * P + mt]
                        w = wl1_sb
                        dst_tab, r0 = xl1_tab, t * P
                    else:       # xr1 for own slice
                        g = t - NT
                        mt = GN
                        lhs = xsT_sb[:, g * P:g * P + mt]
                        w = wr1_sb
                        dst_tab, r0 = xr1_sl, g * P
                    for half in range(2):
                        ps = aps.tile([P, 1024], dt.float32, tag="ps")
                        for j in range(2):
                            nc.tensor.matmul(
                                out=ps[:mt, j * 512:(j + 1) * 512],
                                lhsT=lhs,
                                rhs=w[:, half * 1024 + j * 512:
                                      half * 1024 + (j + 1) * 512],
                                start=True, stop=True)
                        xsb = asb.tile([P, 1024], dt.bfloat16, tag="xsb")
                        if half == 0:
                            nc.scalar.copy(out=xsb[:mt], in_=ps[:mt])
                        else:
                            nc.vector.tensor_copy(out=xsb[:mt], in_=ps[:mt])
                        nc.sync.dma_start(
                            out=dst_tab[r0:r0 + mt,
                                        half * 1024:(half + 1) * 1024],
                            in_=xsb[:mt])
                # layer-2 edge projections for all chunks (resident)
                NB = (NCH + 15) // 16
                for blk in range(NB):
                    ps2 = aps2.tile([P, 512], dt.float32, tag="ep2")
                    n_in_blk = min(16, NCH - blk * 16)
                    for j in range(n_in_blk):
                        ch = blk * 16 + j
                        nc.tensor.matmul(
                            out=ps2[:, j * D2:(j + 1) * D2],
                            lhsT=eaT_sb[:, ch * P:(ch + 1) * P],
                            rhs=we2_sb[:], start=True, stop=True)
                    nc.vector.tensor_copy(
                        out=ep2_sb[:, blk * 512:blk * 512 + n_in_blk * D2],
                        in_=ps2[:, :n_in_blk * D2])

            # ---------- phase B: layer-1 edge pass ----------
            xr2_tiles = []
            with tc.tile_pool(name="xr2res", bufs=G) as xr2p:
              with (
                tc.tile_pool(name="b_acc", bufs=1, space="PSUM") as accp,  # 4
                tc.tile_pool(name="b_ep", bufs=1, space="PSUM") as epp,    # 2
                tc.tile_pool(name="b_sm", bufs=1, space="PSUM") as smp,    # 1
                tc.tile_pool(name="b_x2", bufs=1, space="PSUM") as x2p,    # 1
                tc.tile_pool(name="b_q", bufs=4) as qp,
                tc.tile_pool(name="b_z", bufs=3) as zp,
                tc.tile_pool(name="b_m", bufs=3) as mp,
                tc.tile_pool(name="b_xls", bufs=3) as xlsp,
                tc.tile_pool(name="b_sc", bufs=4) as scp,
                tc.tile_pool(name="b_fin", bufs=2) as finp,
              ):
                for g in range(G):
                    acc = accp.tile([P, D1], dt.float32, tag="acc")
                    den = smp.tile([P, 8], dt.float32, tag="sm")
                    for ch in range(CH):
                        chb = g * CH + ch
                        e0 = chb * P
                        q = qp.tile([P, D1], dt.bfloat16, tag="q")
                        nc.gpsimd.indirect_dma_start(
                            out=q[:], out_offset=None, in_=xl1_tab[:],
                            in_offset=bass.IndirectOffsetOnAxis(
                                ap=srci_sb[:, chb:chb + 1], axis=0))
                        nc.gpsimd.indirect_dma_start(
                            out=q[:], out_offset=None, in_=xr1_sl[:],
                            in_offset=bass.IndirectOffsetOnAxis(
                                ap=dstpi_sb[:, chb:chb + 1], axis=0),
                            compute_op=ALU.add)
                        z = zp.tile([P, D1], dt.bfloat16, tag="z")
                        m = mp.tile([P, D1], dt.bfloat16, tag="m")
                        logit = scp.tile([P, 8], dt.float32, tag="lg")
                        ex = scp.tile([P, 8], dt.float32, tag="ex")
                        exb = scp.tile([P, 8], dt.bfloat16, tag="exb")
                        xls = xlsp.tile([P, D1], dt.bfloat16, tag="xls")
                        for half in range(2):
                            c0 = half * 1024
                            ep = epp.tile([P, 1024], dt.float32, tag="ep")
                            for j in range(2):
                                nc.tensor.matmul(
                                    out=ep[:, j * 512:(j + 1) * 512],
                                    lhsT=eaT_sb[:, e0:e0 + P],
                                    rhs=we1_sb[:, c0 + j * 512:
                                               c0 + (j + 1) * 512],
                                    start=True, stop=True)
                            nc.vector.tensor_add(
                                out=z[:, c0:c0 + 1024],
                                in0=q[:, c0:c0 + 1024], in1=ep[:])
                            nc.scalar.activation(
                                out=m[:, c0:c0 + 1024],
                                in_=z[:, c0:c0 + 1024],
                                func=AF.Prelu, alpha=NEG)
                        for h in range(H1):
                            nc.vector.scalar_tensor_tensor(
                                out=z[:, h * C1:(h + 1) * C1],
                                in0=m[:, h * C1:(h + 1) * C1], scalar=1.0,
                                in1=att1_sb[:, h * C1:(h + 1) * C1],
                                op0=ALU.mult, op1=ALU.mult,
                                accum_out=logit[:, h:h + 1])
                        nc.scalar.activation(out=ex[:], in_=logit[:],
                                             func=AF.Exp)
                        nc.scalar.copy(out=exb[:], in_=ex[:])
                        for h in range(H1):
                            nc.vector.tensor_scalar(
                                out=xls[:, h * C1:(h + 1) * C1],
                                in0=q[:, h * C1:(h + 1) * C1],
                                scalar1=ex[:, h:h + 1], scalar2=None,
                                op0=ALU.mult)
                        for j in range(4):
                            nc.tensor.matmul(
                                out=acc[:, j * 512:(j + 1) * 512],
                                lhsT=s01_sb[:, e0:e0 + P],
                                rhs=xls[:, j * 512:(j + 1) * 512],
                                start=(ch == 0), stop=(ch == CH - 1))
                        nc.tensor.matmul(
                            out=den[:], lhsT=s01_sb[:, e0:e0 + P], rhs=exb[:],
                            start=(ch == 0), stop=(ch == CH - 1))

                    # ---- group finalize ----
                    dn = scp.tile([P, 8], dt.float32, tag="dn")
                    nc.vector.reciprocal(out=dn[:], in_=den[:])
                    xr_g = finp.tile([P, D1], dt.bfloat16, tag="xrg")
                    nc.sync.dma_start(out=xr_g[:],
                                      in_=xr1_sl[g * P:(g + 1) * P, :])
                    hs = finp.tile([P, D1], dt.bfloat16, tag="hs")
                    for h in range(H1):
                        nc.vector.scalar_tensor_tensor(
                            out=hs[:, h * C1:(h + 1) * C1],
                            in0=acc[:, h * C1:(h + 1) * C1],
                            scalar=dn[:, h:h + 1],
                            in1=xr_g[:, h * C1:(h + 1) * C1],
                            op0=ALU.mult, op1=ALU.subtract)
                    hr = finp.tile([P, D1], dt.bfloat16, tag="hr")
                    nc.scalar.activation(out=hr[:], in_=hs[:], func=AF.Relu)
                    x2ps = x2p.tile([P, 2 * D2], dt.float32, tag="x2")
                    for half in range(2):
                        tp = smp.tile([P, 1024], dt.bfloat16, tag="sm")
                        for k in range(8):
                            kk = half * 8 + k
                            nc.tensor.transpose(
                                out=tp[:, k * P:(k + 1) * P],
                                in_=hr[:, kk * P:(kk + 1) * P],
                                identity=id_sb[:])
                        hT = finp.tile([P, 1024], dt.bfloat16, tag="hT")
                        nc.vector.tensor_copy(out=hT[:], in_=tp[:])
                        for k in range(8):
                            kk = half * 8 + k
                            nc.tensor.matmul(
                                out=x2ps[:, 0:2 * D2],
                                lhsT=hT[:, k * P:(k + 1) * P],
                                rhs=wlr2_sb[:, kk * 2 * D2:(kk + 1) * 2 * D2],
                                start=(kk == 0), stop=(kk == 15))
                    x2sb = finp.tile([P, 2 * D2], dt.float32, tag="x2sb")
                    nc.vector.tensor_copy(out=x2sb[:], in_=x2ps[:])
                    nc.sync.dma_start(out=xl2_own[g * GN:(g + 1) * GN, :],
                                      in_=x2sb[:GN, 0:D2])
                    nc.sync.dma_start(out=xr2_tab[g * P:(g + 1) * P, :],
                                      in_=x2sb[:, D2:2 * D2])
                    xr2_res = xr2p.tile([P, D2], dt.float32, tag="xr2")
                    nc.vector.tensor_copy(out=xr2_res[:], in_=x2sb[:, D2:])
                    xr2_tiles.append(xr2_res)

              # ---------- AllGather of xl2 ----------
              if True:
                nc.gpsimd.collective_compute(
                    "AllGather", ALU.bypass, replica_groups=RG,
                    ins=[xl2_own[:]], outs=[xl2_all[:]])

                # ---------- phase C: layer-2 edge pass ----------
                with (
                    tc.tile_pool(name="c_ps", bufs=2, space="PSUM") as cps,
                    tc.tile_pool(name="c_q", bufs=NCH) as cqp,
                    tc.tile_pool(name="c_sb", bufs=4) as csb,
                ):
                    # prefetch local xr2[dst] for every chunk (no AG dep)
                    q2_tiles = []
                    for chb in range(NCH):
                        q2 = cqp.tile([P, D2], dt.float32, tag="q2")
                        nc.gpsimd.indirect_dma_start(
                            out=q2[:], out_offset=None, in_=xr2_tab[:],
                            in_offset=bass.IndirectOffsetOnAxis(
                                ap=dstpi_sb[:, chb:chb + 1], axis=0))
                        q2_tiles.append(q2)
                    for g in range(G):
                        acc2 = cps.tile([P, D2 + 1], dt.float32, tag="a2")
                        for ch in range(CH):
                            chb = g * CH + ch
                            e0 = chb * P
                            q2 = q2_tiles[chb]
                            nc.gpsimd.indirect_dma_start(
                                out=q2[:], out_offset=None, in_=xl2_all[:],
                                in_offset=bass.IndirectOffsetOnAxis(
                                    ap=srci_sb[:, chb:chb + 1], axis=0),
                                compute_op=ALU.add)
                            z2 = csb.tile([P, D2], dt.float32, tag="z2")
                            nc.vector.tensor_add(
                                out=z2[:], in0=q2[:],
                                in1=ep2_sb[:, chb * D2:(chb + 1) * D2])
                            m2 = csb.tile([P, D2], dt.bfloat16, tag="m2")
                            nc.scalar.activation(out=m2[:], in_=z2[:],
                                                 func=AF.Prelu, alpha=NEG)
                            lg2 = csb.tile([P, 1], dt.float32, tag="lg2")
                            sc2 = csb.tile([P, D2], dt.bfloat16, tag="sc2")
                            nc.vector.scalar_tensor_tensor(
                                out=sc2[:], in0=m2[:], scalar=1.0,
                                in1=att2_sb[:], op0=ALU.mult, op1=ALU.mult,
                                accum_out=lg2[:, :1])
                            xls2 = csb.tile([P, D2 + 1], dt.bfloat16,
                                            tag="xls2")
                            ex2 = csb.tile([P, 1], dt.float32, tag="ex2")
                            nc.scalar.activation(out=ex2[:], in_=lg2[:],
                                                 func=AF.Exp)
                            nc.scalar.copy(out=xls2[:, D2:D2 + 1], in_=ex2[:])
                            nc.vector.tensor_scalar(
                                out=xls2[:, :D2], in0=q2[:],
                                scalar1=ex2[:, :1], scalar2=None,
                                op0=ALU.mult)
                            nc.tensor.matmul(
                                out=acc2[:], lhsT=s01_sb[:, e0:e0 + P],
                                rhs=xls2[:],
                                start=(ch == 0), stop=(ch == CH - 1))
                        d2 = csb.tile([P, 1], dt.float32, tag="d2")
                        nc.vector.reciprocal(out=d2[:], in_=acc2[:, D2:D2 + 1])
                        o2 = csb.tile([P, D2], dt.float32, tag="o2")
                        nc.vector.scalar_tensor_tensor(
                            out=o2[:], in0=acc2[:, :D2], scalar=d2[:, :1],
                            in1=xr2_tiles[g], op0=ALU.mult, op1=ALU.subtract)
                        orl = csb.tile([P, D2], dt.float32, tag="orl")
                        nc.vector.tensor_scalar(
                            out=orl[:], in0=o2[:], scalar1=0.0, scalar2=None,
                            op0=ALU.max)
                        nc.sync.dma_start(out=out[g * GN:(g + 1) * GN, :],
                                          in_=orl[:GN])

    nc.compile()
    return nc


def _prep_inputs(x, edge_index, edge_attr, Wl1, bl1, Wr1, br1, We1, att1, b1,
                 Wl2, bl2, Wr2, br2, We2, att2, b2):
    for b in (bl1, br1, b1, bl2, br2, b2):
        assert not np.any(np.asarray(b)), "nonzero biases not implemented"

    src = np.asarray(edge_index[0], dtype=np.int64)
    dst = np.asarray(edge_index[1], dtype=np.int64)
    ea = np.asarray(edge_attr, dtype=np.float32)

    # PyG fill_value='mean' self loops, computed host-side
    cnt = np.bincount(dst, minlength=N).astype(np.float32)
    ssum = np.zeros((N, F), np.float32)
    np.add.at(ssum, dst, ea)
    self_attr = ssum / np.maximum(cnt, 1.0)[:, None]

    order = np.argsort(dst, kind="stable")
    s_src, s_dst, s_ea = src[order], dst[order], ea[order]
    bounds = np.searchsorted(s_dst, np.arange(0, N + GN, GN))
    cnts = np.diff(bounds)                       # real edges per group (80,)
    CH = int(np.max((cnts + GN + P - 1) // P))   # incl. GN self edges
    NCH = G * CH
    L = NCH * P

    x = np.asarray(x, dtype=np.float32)
    common = {
        "xT": x.T.astype(BF16),
        "wl1": np.asarray(Wl1, np.float32).astype(BF16),
        "wr1": np.asarray(Wr1, np.float32).astype(BF16),
        "we1": np.asarray(We1, np.float32).astype(BF16),
        "att1r": np.tile(np.asarray(att1, np.float32).reshape(1, D1),
                         (P, 1)).astype(BF16),
        "wlr2": np.concatenate([
                    np.asarray(Wl2, np.float32).reshape(16, P, D2),
                    np.asarray(Wr2, np.float32).reshape(16, P, D2)],
                    axis=2).transpose(1, 0, 2).reshape(P, 32 * D2)
                .astype(BF16),
        "we2": np.asarray(We2, np.float32).astype(BF16),
        "att2r": np.tile(np.asarray(att2, np.float32).reshape(1, D2),
                         (P, 1)).astype(BF16),
        "ident": np.eye(P, dtype=np.float32).astype(BF16),
    }

    in_maps = []
    for k in range(M):
        base_node = k * NPC
        ea_c = np.zeros((L, F), np.float32)
        s01_c = np.zeros((L, P), np.float32)
        srci_c = np.zeros((L,), np.int32)
        dstpi_c = np.zeros((L,), np.int32)
        for g in range(G):
            gb = base_node + g * GN
            lo, hi = bounds[k * G + g], bounds[k * G + g + 1]
            cnt_g = hi - lo
            tot = cnt_g + GN
            assert tot <= CH * P
            o0 = g * CH * P
            sl = np.arange(o0, o0 + tot)
            ea_c[sl[:cnt_g]] = s_ea[lo:hi]
            ea_c[sl[cnt_g:]] = self_attr[gb:gb + GN]
            dl = np.concatenate([(s_dst[lo:hi] - gb), np.arange(GN)])
            s01_c[sl, dl] = 1.0
            srci_c[sl] = np.concatenate([s_src[lo:hi], np.arange(gb, gb + GN)])
            dstpi_c[sl] = g * P + dl
        im = dict(common)
        im["xsT"] = np.ascontiguousarray(
            np.pad(x[base_node:base_node + NPC].T.reshape(F, G, GN),
                   ((0, 0), (0, 0), (0, P - GN))).reshape(F, G * P)).astype(BF16)
        im["eaT"] = np.ascontiguousarray(ea_c.T).astype(BF16)
        # [p, ch*128+d] layout: edge slot p of chunk ch
        im["s01T"] = np.ascontiguousarray(
            s01_c.reshape(NCH, P, P).transpose(1, 0, 2)
            .reshape(P, L)).astype(BF16)
        im["srci"] = np.ascontiguousarray(srci_c.reshape(NCH, P).T)
        im["dstpi"] = np.ascontiguousarray(dstpi_c.reshape(NCH, P).T)
        in_maps.append(im)
    return in_maps, CH


_PROG_CACHE = {}


def _get_program(CH):
    if CH not in _PROG_CACHE:
        _PROG_CACHE[CH] = _build_program(CH)
    return _PROG_CACHE[CH]


def run(inputs, trace=False, tmpdir=None):
    in_maps, CH = _prep_inputs(**inputs)
    nc = _get_program(CH)
    res = run_bass_kernel_spmd(nc, in_maps, list(range(M)), trace=trace,
                               tmpdir=tmpdir)
    outp = np.concatenate([res.results[k]["out"] for k in range(M)], axis=0)
    return outp.astype(np.float32), res


def kernel(**inputs):
    outp, _ = run(inputs)
    return outp


# revision 16
# speedup vs baseline: 1.4327x; 1.0472x over previous
"""GATv2 2-layer GNN on 8 Trainium2 NeuronCores (Bass/Tile, edge-parallel).

Sharding: edges sorted by dst node, dst-range sharded across 8 cores
(core k owns dst nodes [1250k, 1250(k+1))), so the per-dst segment
softmax and aggregation are fully core-local. Self-loop edge attrs
(mean of incoming) are precomputed host-side and folded into the edge
stream. Layer-1 node projections are computed replicated (xl1 for all
nodes; xr1 for the own slice). Between layers only the 32-wide xl2
projection is exchanged with a single AllGather (xr2 is dst-local).

Per-edge-chunk pipeline (128 edges):
  gpsimd: q = xl1[src] (+)DMA xr1[dst]          (fused add-gather, bf16)
  tensor: ep = ea @ We1 into PSUM (2x 1024-wide matmuls)
  vector: z = q + ep ; 8x logit-accum STT ; 8x xls = q*ex
  scalar: m = LeakyRelu(z) ; ex = Exp(logits)
  tensor: acc += s01^T @ xls ; den += s01^T @ ex
Aggregation identity: sum_e alpha*(xl+xr) = out[d] + xr[d] (softmax
sums to 1), so the group finalize computes h = relu(acc/den - xr[d])
and xl never needs to be gathered standalone.
"""
import sys
sys.path.insert(0, "/opt/trn_rl_repo")

import numpy as np
import ml_dtypes

import concourse.bass as bass
import concourse.bacc as bacc
import concourse.tile as tile
from concourse import mybir
from concourse.bass_utils import run_bass_kernel_spmd

BF16 = ml_dtypes.bfloat16

N, E, F = 10000, 80000, 128
H1, C1 = 8, 256
D1 = H1 * C1          # 2048
D2 = 32               # layer-2 out (1 head)
NEG = 0.2
M = 8                 # cores
NPC = N // M          # 1250 nodes per core
GN = 125              # dst nodes per group
G = NPC // GN         # 10 groups per core
P = 128

dt = mybir.dt


def _build_program(CH):
    """Build the SPMD Bass program. CH = chunks per group (self edges incl)."""
    NCH = G * CH          # chunks per core
    L = NCH * P           # edge-stream length per core
    nc = bacc.Bacc("TRN2", target_bir_lowering=False, debug=False, num_devices=M)

    ei = {}
    def EIN(name, shape, dtype):
        ei[name] = nc.dram_tensor(name, list(shape), dtype, kind="ExternalInput")
        return ei[name]

    xT    = EIN("xT",    (P, N),       dt.bfloat16)   # x transposed (replicated)
    xsT   = EIN("xsT",   (P, G * P),   dt.bfloat16)   # own-slice cols, group padded
    wl1   = EIN("wl1",   (F, D1),      dt.bfloat16)
    wr1   = EIN("wr1",   (F, D1),      dt.bfloat16)
    we1   = EIN("we1",   (F, D1),      dt.bfloat16)
    att1r = EIN("att1r", (P, D1),      dt.bfloat16)   # att1 row-replicated (raw)
    wlr2  = EIN("wlr2",  (P, 16 * 2 * D2), dt.bfloat16)  # [p, k*64+c]: Wl2|Wr2
    we2   = EIN("we2",   (F, D2),      dt.bfloat16)
    att2r = EIN("att2r", (P, D2),      dt.bfloat16)
    ident = EIN("ident", (P, P),       dt.bfloat16)   # identity (PE transpose)
    eaT   = EIN("eaT",   (F, L),       dt.bfloat16)   # edge_attr^T, sorted+padded
    s01T  = EIN("s01T",  (P, L),       dt.bfloat16)   # [p, ch*128+d]: edge p of
                                                      # chunk ch one-hot on dst d
    srci  = EIN("srci",  (P, NCH),     dt.int32)      # global src id per slot
    dstpi = EIN("dstpi", (P, NCH),     dt.int32)      # group-padded local dst id
    dstli = EIN("dstli", (P, NCH),     dt.int32)      # within-group local dst id

    out = nc.dram_tensor("out", [NPC, D2], dt.float32, kind="ExternalOutput")

    # DRAM scratch
    xl1_tab = nc.dram_tensor("xl1_tab", [N, D1], dt.bfloat16)
    xr1_sl  = nc.dram_tensor("xr1_sl", [G * P, D1], dt.bfloat16)
    xl2_own = nc.dram_tensor("xl2_own", [NPC, D2], dt.float32)
    xr2_tabs = [nc.dram_tensor(f"xr2_tab{g}", [P, D2], dt.float32)
                for g in range(G)]
    xl2_all = nc.dram_tensor("xl2_all", [N, D2], dt.float32, addr_space="Shared")
    dum0    = nc.dram_tensor("dum0", [1, 8], dt.bfloat16)
    dumAG   = nc.dram_tensor("dumAG", [M, 8], dt.bfloat16, addr_space="Shared")
    dumsink = nc.dram_tensor("dumsink", [1, 8], dt.bfloat16)

    AF = mybir.ActivationFunctionType
    ALU = mybir.AluOpType
    RG = [list(range(M))]

    with tile.TileContext(nc) as tc:
        with tc.tile_pool(name="consts", bufs=1) as cp:
            def CONST(name, src, shape, dtype):
                t = cp.tile(list(shape), dtype, name=name)
                nc.sync.dma_start(out=t[:], in_=src[:])
                return t

            xT_sb   = CONST("xT_sb", xT, (P, N), dt.bfloat16)
            xsT_sb  = CONST("xsT_sb", xsT, (P, G * P), dt.bfloat16)
            wl1_sb  = CONST("wl1_sb", wl1, (F, D1), dt.bfloat16)
            wr1_sb  = CONST("wr1_sb", wr1, (F, D1), dt.bfloat16)
            we1_sb  = CONST("we1_sb", we1, (F, D1), dt.bfloat16)
            att1_sb = CONST("att1_sb", att1r, (P, D1), dt.bfloat16)
            wlr2_sb = CONST("wlr2_sb", wlr2, (P, 32 * D2), dt.bfloat16)
            we2_sb  = CONST("we2_sb", we2, (F, D2), dt.bfloat16)
            att2_sb = CONST("att2_sb", att2r, (P, D2), dt.bfloat16)
            id_sb   = CONST("id_sb", ident, (P, P), dt.bfloat16)
            eaT_sb  = CONST("eaT_sb", eaT, (F, L), dt.bfloat16)
            s01_sb  = CONST("s01_sb", s01T, (P, L), dt.bfloat16)
            srci_sb = CONST("srci_sb", srci, (P, NCH), dt.int32)
            dstpi_sb = CONST("dstpi_sb", dstpi, (P, NCH), dt.int32)
            dstli_sb = CONST("dstli_sb", dstli, (P, NCH), dt.int32)
            ep2_sb = cp.tile([P, NCH * D2], dt.bfloat16, name="ep2_sb")

            # warm-up collective: absorbs CC cold-start under phase A/B
            nc.sync.dma_start(out=dum0[:], in_=xT[0:1, 0:8])
            nc.gpsimd.collective_compute(
                "AllGather", ALU.bypass, replica_groups=RG,
                ins=[dum0[:]], outs=[dumAG[:]])
            nc.sync.dma_start(out=dumsink[:], in_=dumAG[0:1])

            # ---------- phase A: node projections + ep2 precompute ----------
            with (
                tc.tile_pool(name="a_ps", bufs=3, space="PSUM") as aps,
                tc.tile_pool(name="a_ps2", bufs=1, space="PSUM") as aps2,
                tc.tile_pool(name="a_sb", bufs=4) as asb,
            ):
                NT = (N + P - 1) // P
                for t in range(NT + G):
                    if t < NT:  # xl1 for ALL nodes (replicated compute)
                        mt = min(P, N - t * P)
                        lhs = xT_sb[:, t * P:t * P + mt]
                        w = wl1_sb
                        dst_tab, r0 = xl1_tab, t * P
                    else:       # xr1 for own slice
                        g = t - NT
                        mt = GN
                        lhs = xsT_sb[:, g * P:g * P + mt]
                        w = wr1_sb
                        dst_tab, r0 = xr1_sl, g * P
                    for half in range(2):
                        ps = aps.tile([P, 1024], dt.float32, tag="ps")
                        for j in range(2):
                            nc.tensor.matmul(
                                out=ps[:mt, j * 512:(j + 1) * 512],
                                lhsT=lhs,
                                rhs=w[:, half * 1024 + j * 512:
                                      half * 1024 + (j + 1) * 512],
                                start=True, stop=True)
                        xsb = asb.tile([P, 1024], dt.bfloat16, tag="xsb")
                        if half == 0:
                            nc.scalar.copy(out=xsb[:mt], in_=ps[:mt])
                        else:
                            nc.vector.tensor_copy(out=xsb[:mt], in_=ps[:mt])
                        nc.sync.dma_start(
                            out=dst_tab[r0:r0 + mt,
                                        half * 1024:(half + 1) * 1024],
                            in_=xsb[:mt])
                # layer-2 edge projections for all chunks (resident)
                NB = (NCH + 15) // 16
                for blk in range(NB):
                    ps2 = aps2.tile([P, 512], dt.float32, tag="ep2")
                    n_in_blk = min(16, NCH - blk * 16)
                    for j in range(n_in_blk):
                        ch = blk * 16 + j
                        nc.tensor.matmul(
                            out=ps2[:, j * D2:(j + 1) * D2],
                            lhsT=eaT_sb[:, ch * P:(ch + 1) * P],
                            rhs=we2_sb[:], start=True, stop=True)
                    nc.vector.tensor_copy(
                        out=ep2_sb[:, blk * 512:blk * 512 + n_in_blk * D2],
                        in_=ps2[:, :n_in_blk * D2])

            # ---------- phase B: layer-1 edge pass ----------
            xr2_tiles = []
            q2_tiles = []
            with (tc.tile_pool(name="xr2res", bufs=G) as xr2p,
                  tc.tile_pool(name="c_q", bufs=NCH) as cqp):
              with (
                tc.tile_pool(name="b_acc", bufs=1, space="PSUM") as accp,  # 4
                tc.tile_pool(name="b_ep", bufs=1, space="PSUM") as epp,    # 2
                tc.tile_pool(name="b_sm", bufs=1, space="PSUM") as smp,    # 1
                tc.tile_pool(name="b_x2", bufs=1, space="PSUM") as x2p,    # 1
                tc.tile_pool(name="b_q", bufs=4) as qp,
                tc.tile_pool(name="b_z", bufs=3) as zp,
                tc.tile_pool(name="b_m", bufs=3) as mp,
                tc.tile_pool(name="b_xls", bufs=3) as xlsp,
                tc.tile_pool(name="b_sc", bufs=4) as scp,
                tc.tile_pool(name="b_fin", bufs=2) as finp,
              ):
                for g in range(G):
                    acc = accp.tile([P, D1], dt.float32, tag="acc")
                    den = smp.tile([P, 8], dt.float32, tag="sm")
                    for ch in range(CH):
                        chb = g * CH + ch
                        e0 = chb * P
                        q = qp.tile([P, D1], dt.bfloat16, tag="q")
                        nc.gpsimd.indirect_dma_start(
                            out=q[:], out_offset=None, in_=xl1_tab[:],
                            in_offset=bass.IndirectOffsetOnAxis(
                                ap=srci_sb[:, chb:chb + 1], axis=0))
                        nc.gpsimd.indirect_dma_start(
                            out=q[:], out_offset=None, in_=xr1_sl[:],
                            in_offset=bass.IndirectOffsetOnAxis(
                                ap=dstpi_sb[:, chb:chb + 1], axis=0),
                            compute_op=ALU.add)
                        z = zp.tile([P, D1], dt.bfloat16, tag="z")
                        m = mp.tile([P, D1], dt.bfloat16, tag="m")
                        logit = scp.tile([P, 8], dt.float32, tag="lg")
                        ex = scp.tile([P, 8], dt.float32, tag="ex")
                        exb = scp.tile([P, 8], dt.bfloat16, tag="exb")
                        xls = xlsp.tile([P, D1], dt.bfloat16, tag="xls")
                        for half in range(2):
                            c0 = half * 1024
                            ep = epp.tile([P, 1024], dt.float32, tag="ep")
                            for j in range(2):
                                nc.tensor.matmul(
                                    out=ep[:, j * 512:(j + 1) * 512],
                                    lhsT=eaT_sb[:, e0:e0 + P],
                                    rhs=we1_sb[:, c0 + j * 512:
                                               c0 + (j + 1) * 512],
                                    start=True, stop=True)
                            nc.vector.tensor_add(
                                out=z[:, c0:c0 + 1024],
                                in0=q[:, c0:c0 + 1024], in1=ep[:])
                            nc.scalar.activation(
                                out=m[:, c0:c0 + 1024],
                                in_=z[:, c0:c0 + 1024],
                                func=AF.Prelu, alpha=NEG)
                        for h in range(H1):
                            nc.vector.scalar_tensor_tensor(
                                out=z[:, h * C1:(h + 1) * C1],
                                in0=m[:, h * C1:(h + 1) * C1], scalar=1.0,
                                in1=att1_sb[:, h * C1:(h + 1) * C1],
                                op0=ALU.mult, op1=ALU.mult,
                                accum_out=logit[:, h:h + 1])
                        nc.scalar.activation(out=ex[:], in_=logit[:],
                                             func=AF.Exp)
                        nc.scalar.copy(out=exb[:], in_=ex[:])
                        for h in range(H1):
                            if h % 2 == 0:
                                nc.vector.tensor_scalar(
                                    out=xls[:, h * C1:(h + 1) * C1],
                                    in0=q[:, h * C1:(h + 1) * C1],
                                    scalar1=ex[:, h:h + 1], scalar2=None,
                                    op0=ALU.mult)
                            else:
                                nc.scalar.activation(
                                    out=xls[:, h * C1:(h + 1) * C1],
                                    in_=q[:, h * C1:(h + 1) * C1],
                                    func=AF.Copy, scale=ex[:, h:h + 1])
                        for j in range(4):
                            nc.tensor.matmul(
                                out=acc[:, j * 512:(j + 1) * 512],
                                lhsT=s01_sb[:, e0:e0 + P],
                                rhs=xls[:, j * 512:(j + 1) * 512],
                                start=(ch == 0), stop=(ch == CH - 1))
                        nc.tensor.matmul(
                            out=den[:], lhsT=s01_sb[:, e0:e0 + P], rhs=exb[:],
                            start=(ch == 0), stop=(ch == CH - 1))

                    # ---- group finalize ----
                    dn = scp.tile([P, 8], dt.float32, tag="dn")
                    nc.vector.reciprocal(out=dn[:], in_=den[:])
                    xr_g = finp.tile([P, D1], dt.bfloat16, tag="xrg")
                    nc.sync.dma_start(out=xr_g[:],
                                      in_=xr1_sl[g * P:(g + 1) * P, :])
                    hs = finp.tile([P, D1], dt.bfloat16, tag="hs")
                    for h in range(H1):
                        nc.vector.scalar_tensor_tensor(
                            out=hs[:, h * C1:(h + 1) * C1],
                            in0=acc[:, h * C1:(h + 1) * C1],
                            scalar=dn[:, h:h + 1],
                            in1=xr_g[:, h * C1:(h + 1) * C1],
                            op0=ALU.mult, op1=ALU.subtract)
                    hr = finp.tile([P, D1], dt.bfloat16, tag="hr")
                    nc.scalar.activation(out=hr[:], in_=hs[:], func=AF.Relu)
                    x2ps = x2p.tile([P, 2 * D2], dt.float32, tag="x2")
                    for half in range(2):
                        tp = smp.tile([P, 1024], dt.bfloat16, tag="sm")
                        for k in range(8):
                            kk = half * 8 + k
                            nc.tensor.transpose(
                                out=tp[:, k * P:(k + 1) * P],
                                in_=hr[:, kk * P:(kk + 1) * P],
                                identity=id_sb[:])
                        hT = finp.tile([P, 1024], dt.bfloat16, tag="hT")
                        nc.vector.tensor_copy(out=hT[:], in_=tp[:])
                        for k in range(8):
                            kk = half * 8 + k
                            nc.tensor.matmul(
                                out=x2ps[:, 0:2 * D2],
                                lhsT=hT[:, k * P:(k + 1) * P],
                                rhs=wlr2_sb[:, kk * 2 * D2:(kk + 1) * 2 * D2],
                                start=(kk == 0), stop=(kk == 15))
                    x2sb = finp.tile([P, 2 * D2], dt.float32, tag="x2sb")
                    nc.vector.tensor_copy(out=x2sb[:], in_=x2ps[:])
                    nc.sync.dma_start(out=xl2_own[g * GN:(g + 1) * GN, :],
                                      in_=x2sb[:GN, 0:D2])
                    nc.sync.dma_start(out=xr2_tabs[g][:],
                                      in_=x2sb[:, D2:2 * D2])
                    xr2_res = xr2p.tile([P, D2], dt.float32, tag="xr2")
                    nc.vector.tensor_copy(out=xr2_res[:], in_=x2sb[:, D2:])
                    xr2_tiles.append(xr2_res)
                    # phase-C xr2[dst] prefetch for this group's chunks:
                    # depends only on xr2_tabs[g], so it runs during phase B
                    for ch in range(CH):
                        chb = g * CH + ch
                        q2 = cqp.tile([P, D2], dt.float32, tag="q2")
                        nc.gpsimd.indirect_dma_start(
                            out=q2[:], out_offset=None, in_=xr2_tabs[g][:],
                            in_offset=bass.IndirectOffsetOnAxis(
                                ap=dstli_sb[:, chb:chb + 1], axis=0))
                        q2_tiles.append(q2)

              # ---------- AllGather of xl2 ----------
              if True:
                nc.gpsimd.collective_compute(
                    "AllGather", ALU.bypass, replica_groups=RG,
                    ins=[xl2_own[:]], outs=[xl2_all[:]])

                # ---------- phase C: layer-2 edge pass ----------
                with (
                    tc.tile_pool(name="c_ps", bufs=2, space="PSUM") as cps,
                    tc.tile_pool(name="c_sb", bufs=4) as csb,
                ):
                    for g in range(G):
                        acc2 = cps.tile([P, D2 + 1], dt.float32, tag="a2")
                        for ch in range(CH):
                            chb = g * CH + ch
                            e0 = chb * P
                            q2 = q2_tiles[chb]
                            nc.gpsimd.indirect_dma_start(
                                out=q2[:], out_offset=None, in_=xl2_all[:],
                                in_offset=bass.IndirectOffsetOnAxis(
                                    ap=srci_sb[:, chb:chb + 1], axis=0),
                                compute_op=ALU.add)
                            z2 = csb.tile([P, D2], dt.float32, tag="z2")
                            nc.vector.tensor_add(
                                out=z2[:], in0=q2[:],
                                in1=ep2_sb[:, chb * D2:(chb + 1) * D2])
                            m2 = csb.tile([P, D2], dt.bfloat16, tag="m2")
                            nc.scalar.activation(out=m2[:], in_=z2[:],
                                                 func=AF.Prelu, alpha=NEG)
                            lg2 = csb.tile([P, 1], dt.float32, tag="lg2")
                            sc2 = csb.tile([P, D2], dt.bfloat16, tag="sc2")
                            nc.vector.scalar_tensor_tensor(
                                out=sc2[:], in0=m2[:], scalar=1.0,
                                in1=att2_sb[:], op0=ALU.mult, op1=ALU.mult,
                                accum_out=lg2[:, :1])
                            xls2 = csb.tile([P, D2 + 1], dt.bfloat16,
                                            tag="xls2")
                            ex2 = csb.tile([P, 1], dt.float32, tag="ex2")
                            nc.scalar.activation(out=ex2[:], in_=lg2[:],
                                                 func=AF.Exp)
                            nc.scalar.copy(out=xls2[:, D2:D2 + 1], in_=ex2[:])
                            nc.vector.tensor_scalar(
                                out=xls2[:, :D2], in0=q2[:],
                                scalar1=ex2[:, :1], scalar2=None,
                                op0=ALU.mult)
                            nc.tensor.matmul(
                                out=acc2[:], lhsT=s01_sb[:, e0:e0 + P],
                                rhs=xls2[:],
                                start=(ch == 0), stop=(ch == CH - 1))
                        d2 = csb.tile([P, 1], dt.float32, tag="d2")
                        nc.vector.reciprocal(out=d2[:], in_=acc2[:, D2:D2 + 1])
                        o2 = csb.tile([P, D2], dt.float32, tag="o2")
                        nc.vector.scalar_tensor_tensor(
                            out=o2[:], in0=acc2[:, :D2], scalar=d2[:, :1],
                            in1=xr2_tiles[g], op0=ALU.mult, op1=ALU.subtract)
                        orl = csb.tile([P, D2], dt.float32, tag="orl")
                        nc.vector.tensor_scalar(
                            out=orl[:], in0=o2[:], scalar1=0.0, scalar2=None,
                            op0=ALU.max)
                        nc.sync.dma_start(out=out[g * GN:(g + 1) * GN, :],
                                          in_=orl[:GN])

    nc.compile()
    return nc


def _prep_inputs(x, edge_index, edge_attr, Wl1, bl1, Wr1, br1, We1, att1, b1,
                 Wl2, bl2, Wr2, br2, We2, att2, b2):
    for b in (bl1, br1, b1, bl2, br2, b2):
        assert not np.any(np.asarray(b)), "nonzero biases not implemented"

    src = np.asarray(edge_index[0], dtype=np.int64)
    dst = np.asarray(edge_index[1], dtype=np.int64)
    ea = np.asarray(edge_attr, dtype=np.float32)

    # PyG fill_value='mean' self loops, computed host-side
    cnt = np.bincount(dst, minlength=N).astype(np.float32)
    ssum = np.zeros((N, F), np.float32)
    np.add.at(ssum, dst, ea)
    self_attr = ssum / np.maximum(cnt, 1.0)[:, None]

    order = np.argsort(dst, kind="stable")
    s_src, s_dst, s_ea = src[order], dst[order], ea[order]
    bounds = np.searchsorted(s_dst, np.arange(0, N + GN, GN))
    cnts = np.diff(bounds)                       # real edges per group (80,)
    CH = int(np.max((cnts + GN + P - 1) // P))   # incl. GN self edges
    NCH = G * CH
    L = NCH * P

    x = np.asarray(x, dtype=np.float32)
    common = {
        "xT": x.T.astype(BF16),
        "wl1": np.asarray(Wl1, np.float32).astype(BF16),
        "wr1": np.asarray(Wr1, np.float32).astype(BF16),
        "we1": np.asarray(We1, np.float32).astype(BF16),
        "att1r": np.tile(np.asarray(att1, np.float32).reshape(1, D1),
                         (P, 1)).astype(BF16),
        "wlr2": np.concatenate([
                    np.asarray(Wl2, np.float32).reshape(16, P, D2),
                    np.asarray(Wr2, np.float32).reshape(16, P, D2)],
                    axis=2).transpose(1, 0, 2).reshape(P, 32 * D2)
                .astype(BF16),
        "we2": np.asarray(We2, np.float32).astype(BF16),
        "att2r": np.tile(np.asarray(att2, np.float32).reshape(1, D2),
                         (P, 1)).astype(BF16),
        "ident": np.eye(P, dtype=np.float32).astype(BF16),
    }

    in_maps = []
    for k in range(M):
        base_node = k * NPC
        ea_c = np.zeros((L, F), np.float32)
        s01_c = np.zeros((L, P), np.float32)
        srci_c = np.zeros((L,), np.int32)
        dstpi_c = np.zeros((L,), np.int32)
        for g in range(G):
            gb = base_node + g * GN
            lo, hi = bounds[k * G + g], bounds[k * G + g + 1]
            cnt_g = hi - lo
            tot = cnt_g + GN
            assert tot <= CH * P
            o0 = g * CH * P
            sl = np.arange(o0, o0 + tot)
            ea_c[sl[:cnt_g]] = s_ea[lo:hi]
            ea_c[sl[cnt_g:]] = self_attr[gb:gb + GN]
            dl = np.concatenate([(s_dst[lo:hi] - gb), np.arange(GN)])
            s01_c[sl, dl] = 1.0
            srci_c[sl] = np.concatenate([s_src[lo:hi], np.arange(gb, gb + GN)])
            dstpi_c[sl] = g * P + dl
        im = dict(common)
        im["xsT"] = np.ascontiguousarray(
            np.pad(x[base_node:base_node + NPC].T.reshape(F, G, GN),
                   ((0, 0), (0, 0), (0, P - GN))).reshape(F, G * P)).astype(BF16)
        im["eaT"] = np.ascontiguousarray(ea_c.T).astype(BF16)
        # [p, ch*128+d] layout: edge slot p of chunk ch
        im["s01T"] = np.ascontiguousarray(
            s01_c.reshape(NCH, P, P).transpose(1, 0, 2)
            .reshape(P, L)).astype(BF16)
        im["srci"] = np.ascontiguousarray(srci_c.reshape(NCH, P).T)
        im["dstpi"] = np.ascontiguousarray(dstpi_c.reshape(NCH, P).T)
        im["dstli"] = np.ascontiguousarray((dstpi_c % P).reshape(NCH, P).T)
        in_maps.append(im)
    return in_maps, CH


_PROG_CACHE = {}


def _get_program(CH):
    if CH not in _PROG_CACHE:
        _PROG_CACHE[CH] = _build_program(CH)
    return _PROG_CACHE[CH]


def run(inputs, trace=False, tmpdir=None):
    in_maps, CH = _prep_inputs(**inputs)
    nc = _get_program(CH)
    res = run_bass_kernel_spmd(nc, in_maps, list(range(M)), trace=trace,
                               tmpdir=tmpdir)
    outp = np.concatenate([res.results[k]["out"] for k in range(M)], axis=0)
    return outp.astype(np.float32), res


def kernel(**inputs):
    outp, _ = run(inputs)
    return outp
